# revision 32
# baseline (speedup 1.0000x reference)
"""Trainium2 Bass kernel for nn_BlockAttentionResidual.

Transformer block: RMSNorm -> QKV -> RoPE -> block-diagonal causal attention
(4 blocks of 512) -> o-proj + residual -> RMSNorm -> SwiGLU FFN + residual.
Shapes: x [2, 2048, 2048], 32 heads x 64, inter 4096.

Sharding: 8 cores = (batch 2) x (4 sequence blocks of 512 tokens). The
attention mask is block-diagonal causal with block size 512, so each core's
512-token slice is fully independent -> no collectives.

On-device layout is "T layout" [feature, token] throughout, because every
matmul contracts the feature dim, which must sit on SBUF partitions for the
PE. Matmuls run in bf16 with fp32 PSUM accumulation; softmax skips the max
subtraction (scores are small); the causal mask is only needed on the
128x128 diagonal chunks of each 512 block; softmax denominators come from a
ones-column appended to V; per-token broadcasts across partitions are done
as rank-1 outer-product matmuls on the PE.

v2: the v-projection and attention are software-pipelined: v-proj for head
group g runs interleaved (one emission "slot" at a time) with the attention
chains for head group g-1, so the PE chews dense v-proj matmuls while the
scalar engine runs the softmax exps that the attention matmuls wait on.
PSUM's 8 banks are hand-allocated via single-buffer pool tags. Softmax exps
for a head pair run as one scalar op over a 2-bank PSUM super-tile; the
softmax reciprocal runs as per-head Ln (doubling as the cross-partition
collect) + one batched Exp per 4-head group.
"""

import math
from contextlib import ExitStack

import ml_dtypes
import numpy as np

import concourse.bass as bass
import concourse.mybir as mybir
import concourse.tile as tile
from concourse.bass_utils import run_bass_kernel_spmd
from concourse.vector_clock import ScopedClock

F32 = mybir.dt.float32
BF16 = mybir.dt.bfloat16
NPBF16 = ml_dtypes.bfloat16

EPS = 1e-5
ROPE_THETA = 10000.0


# --- workaround: this walrus build allows only one sem wait per CTRL-queue
# instruction (Drain/NoOp), but Tile's tail drain aggregates every
# outstanding wait onto a single SP Drain. Spread them over SP NOPs.
def _patched_drain_and_barrier(self, tick_clock, wait_clock):
    nop_inst = self.nc.sync.nop(nofuse=True)
    wait_clock.add_sem_waits(
        nop_inst.ins, ScopedClock({None: tick_clock.global_clock})
    )
    si = nop_inst.ins.sync_info
    waits = list(si.on_wait) if si is not None else []
    if len(waits) > 1:
        si.on_wait = waits[:1]
        for w in waits[1:]:
            n2 = self.nc.sync.nop(nofuse=True)
            if n2.ins.sync_info is None:
                n2.ins.sync_info = mybir.SyncInfo(on_wait=[w], on_update=[])
            else:
                n2.ins.sync_info.on_wait = [w]
    self.nc.sync.drain()
    self.nc.all_engine_barrier()
    assert self.sems is not None
    popped = self.nc._tile_sem_poison_stack.pop()
    assert popped is self._sem_poison
    self.nc.clear_and_free_semaphores(list(self.sems.allocated().values()))
    self.nc.all_engine_barrier()


tile.TileContext._drain_and_barrier = _patched_drain_and_barrier


def _split_excess_waits(nc, maxw=1):
    """This walrus build rejects instructions carrying more than one sync
    wait; hoist extras onto single-wait NOPs queued just before on the same
    engine."""
    fn = nc.m.functions[0]
    for bb in fn.blocks:
        out = []
        changed = False
        for inst in bb.instructions:
            si = getattr(inst, "sync_info", None)
            waits = list(si.on_wait) if si is not None else []
            if len(waits) > maxw:
                changed = True
                for w in waits[:-maxw]:
                    nop = mybir.InstNoOp(
                        name=nc.get_next_instruction_name(), ins=[], outs=[])
                    nop.engine = inst.engine
                    nop.sync_info = mybir.SyncInfo(on_wait=[w], on_update=[])
                    out.append(nop)
                si.on_wait = waits[-maxw:]
            out.append(inst)
        if changed:
            bb.instructions = out


class Cfg:
    def __init__(self, T=512, C=2048, H=32, D=64, I=4096):
        self.T = T          # tokens per core (one attention block)
        self.C = C          # hidden
        self.H = H          # heads
        self.D = D          # head dim (must be 64)
        self.I = I          # ffn inner
        assert D == 64 and C == H * D
        assert T % 128 == 0 and C % 128 == 0 and I % 128 == 0


def build_program(cfg: Cfg):
    T, C, H, D, I = cfg.T, cfg.C, cfg.H, cfg.D, cfg.I
    CT = C // 128            # hidden tiles
    KT = T // 128            # token chunks (and attention k-chunks)
    QKN = 2 * C              # q+k feature rows
    ICH = I // 128           # ffn inner chunks
    scale = 1.0 / math.sqrt(D)
    OG = 8                   # psum group width for dense matmul phases

    nc = bass.Bass("TRN2", target_bir_lowering=False, debug=False)

    xT = nc.dram_tensor("xT", (C, T), F32, kind="ExternalInput").ap()
    xbT = nc.dram_tensor("xbT", (C, T), BF16, kind="ExternalInput").ap()
    w_qkT = nc.dram_tensor("w_qkT", (C, QKN), BF16, kind="ExternalInput").ap()
    w_vT = nc.dram_tensor("w_vT", (C, C), BF16, kind="ExternalInput").ap()
    w_oT = nc.dram_tensor("w_oT", (C, C), BF16, kind="ExternalInput").ap()
    w_upT = nc.dram_tensor("w_upT", (C, 2 * I), BF16, kind="ExternalInput").ap()
    w_downT = nc.dram_tensor("w_downT", (I, C), BF16, kind="ExternalInput").ap()
    nw1 = nc.dram_tensor("nw1", (1, C), BF16, kind="ExternalInput").ap()
    nw2 = nc.dram_tensor("nw2", (1, C), BF16, kind="ExternalInput").ap()
    cosT2 = nc.dram_tensor("cosT2", (128, T), BF16, kind="ExternalInput").ap()
    nsinT2 = nc.dram_tensor("nsinT2", (128, T), BF16, kind="ExternalInput").ap()
    trimask = nc.dram_tensor("trimask", (128, 128), BF16, kind="ExternalInput").ap()
    pswap = nc.dram_tensor("pswap", (128, 128), BF16, kind="ExternalInput").ap()
    outT = nc.dram_tensor("outT", (C, T), F32, kind="ExternalOutput").ap()

    with tile.TileContext(nc) as tc, ExitStack() as ctx:
        consts = ctx.enter_context(tc.tile_pool(name="consts", bufs=1))
        # xt slots also serve x2 (x dies at the o-proj residual add); +2
        # rolling slots so the x2 alloc never waits on its own free.
        xt_pool = ctx.enter_context(tc.tile_pool(name="xt", bufs=CT + 2))
        ht_pool = ctx.enter_context(tc.tile_pool(name="ht", bufs=CT))
        qk_pool = ctx.enter_context(
            tc.tile_pool(name="qk", bufs=max(2 * CT, ICH)))
        v_pool = ctx.enter_context(tc.tile_pool(name="v", bufs=KT))
        ctx_pool = ctx.enter_context(tc.tile_pool(name="ctx", bufs=CT))
        wsl_pool = ctx.enter_context(tc.tile_pool(name="wsl", bufs=8))
        tr_pool = ctx.enter_context(tc.tile_pool(name="tr", bufs=4))
        e_pool = ctx.enter_context(tc.tile_pool(name="e", bufs=4))
        sm_pool = ctx.enter_context(tc.tile_pool(name="sm", bufs=2))
        rsrc_pool = ctx.enter_context(tc.tile_pool(name="rsrc", bufs=9))
        # PSUM: 8 banks, hand-allocated. tags a0..a3/c0/c1 are one bank
        # ([128,512] f32); tag S is a two-bank super-tile ([128,1024]).
        ps_pool = ctx.enter_context(
            tc.tile_pool(name="ps", bufs=1, space="PSUM"))

        _nm = [0]

        def named(base):
            _nm[0] += 1
            return f"{base}{_nm[0]}"

        def ps1(tag):
            return ps_pool.tile([128, T], F32, tag=tag, bufs=1,
                                name=named("ps"))

        def ps2():
            return ps_pool.tile([128, 2 * T], F32, tag="S", bufs=1,
                                name=named("psS"))

        def psC():
            return ps_pool.tile([128, 2 * T], F32, tag="C", bufs=1,
                                name=named("psC"))

        SET_A = ["a0", "a1", "a2", "a3"]

        def dense_group(parity):
            """4 psum [128,T] views for a dense og-group: even parity uses
            banks a0..a3, odd uses the S super-tile halves + c0/c1. Returns
            (views, tiles-to-keep-alive)."""
            if parity == 0:
                ts = [ps1(t) for t in SET_A]
                return [t[:, :] for t in ts], ts
            s = ps2()
            c = psC()
            # C halves first: at phase junctions they free earlier than S,
            # letting the next group's first matmuls start sooner
            return [c[:, 0:T], c[:, T:2 * T], s[:, 0:T], s[:, T:2 * T]], [s, c]

        # ---- constants
        sb_cos = consts.tile([128, T], BF16)
        nc.sync.dma_start(sb_cos[:], cosT2[:])
        sb_nsin = consts.tile([128, T], BF16)
        nc.sync.dma_start(sb_nsin[:], nsinT2[:])
        sb_tri = consts.tile([128, 128], BF16)
        nc.sync.dma_start(sb_tri[:], trimask[:])
        sb_psw = consts.tile([128, 128], BF16)
        nc.sync.dma_start(sb_psw[:], pswap[:])
        sb_nw = consts.tile([1, 2 * C], BF16)
        nc.sync.dma_start(sb_nw[0:1, 0:C], nw1[:])
        nc.sync.dma_start(sb_nw[0:1, C:2 * C], nw2[:])
        ones_col = consts.tile([128, 1], BF16)
        nc.vector.memset(ones_col[:], 1.0)
        ones_all = consts.tile([128, D], BF16)
        nc.vector.memset(ones_all[:], 1.0)
        eps_t = consts.tile([1, 1], F32)
        nc.vector.memset(eps_t[:], EPS)

        # ---- load a bf16 copy of x for rmsnorm1 (half the critical-path
        # DMA bytes); the fp32 x, needed only for the o-proj residual,
        # streams in later spread across the attention-phase slots. xb
        # tiles ride the qk pool rotation ahead of the qkrot tiles.
        xb = []
        for ci in range(CT):
            t = qk_pool.tile([128, T], BF16, tag="qk", name=named("t"))
            nc.sync.dma_start(t[:], xbT[ci * 128:(ci + 1) * 128, :])
            xb.append(t)
        xt = [xt_pool.tile([128, T], F32, tag="xt", name=named("t"))
              for _ in range(CT)]

        def rmsnorm(x_tiles, nw_row, out_tag, ss_sb=None):
            """x_tiles: CT fp32 [128, T] tiles (T layout) -> bf16 tiles of
            x * nw[c] * rstd[t]. ss_sb: optionally an sbuf [1, T] tile
            already holding the sum of squares."""
            if ss_sb is None:
                ps_ss = psC()
                for ci in range(CT):
                    sq = tr_pool.tile([128, T], BF16, tag="trb0", name=named("t"))
                    nc.scalar.square(sq[:], x_tiles[ci][:])
                    nc.tensor.matmul(
                        ps_ss[0:1, 0:T], ones_col[:], sq[:],
                        start=(ci == 0), stop=(ci == CT - 1),
                    )
                ss_in = ps_ss[0:1, 0:T]
            else:
                ss_in = ss_sb[0:1, :]
            s_sb = sm_pool.tile([1, T], F32, tag="s1", name=named("t"))
            nc.scalar.activation(
                s_sb[:], ss_in, mybir.ActivationFunctionType.Ln,
                bias=eps_t[:], scale=1.0 / C,
            )
            rstd = sm_pool.tile([1, T], BF16, tag="s2", name=named("t"))
            nc.scalar.activation(
                rstd[:], s_sb[:], mybir.ActivationFunctionType.Exp,
                scale=-0.5,
            )
            out = []
            cyc = ["a0", "a1", "a2", "a3"]
            for ci in range(CT):
                ps_sc = ps1(cyc[ci % len(cyc)])
                nc.tensor.matmul(
                    ps_sc[:, :],
                    sb_nw[0:1, nw_row * C + ci * 128:nw_row * C + (ci + 1) * 128],
                    rstd[:],
                    start=True, stop=True,
                )
                h = ht_pool.tile([128, T], BF16, tag=out_tag, name=named("t"))
                nc.vector.tensor_mul(h[:], x_tiles[ci][:], ps_sc[:, :])
                out.append(h)
            return out

        # ---- rmsnorm 1
        ht = rmsnorm(xb, 0, "ht")

        # ---- q/k projection (T layout) + RoPE
        qkrot = [None] * (QKN // 128)
        n_och = QKN // 128
        OGQ = 4
        rope_pend = []

        def flush_rope(n=99):
            # emitted one og-group late (dripped between matmul bursts) so
            # the swap matmul's inputs are ready and the DVE work is spread
            for _ in range(min(n, len(rope_pend))):
                idx, src, mk_ps = rope_pend.pop(0)
                a = tr_pool.tile([128, T], BF16, tag="trb2", bufs=2, name=named("t"))
                nc.vector.tensor_mul(a[:], src[:], sb_cos[:])
                m = tr_pool.tile([128, T], BF16, tag="trb3", bufs=2, name=named("t"))
                nc.vector.tensor_mul(m[:], src[:], sb_nsin[:])
                ps_b = mk_ps()
                nc.tensor.matmul(ps_b, sb_psw[:], m[:],
                                 start=True, stop=True)
                rot = qk_pool.tile([128, T], BF16, tag="qk", name=named("t"))
                nc.vector.tensor_add(rot[:], a[:], ps_b)
                qkrot[idx] = rot

        qk_mkps = [
            [lambda t=t: ps1(t)[:, :] for t in SET_A],
            [lambda: psC()[:, 0:T], lambda: psC()[:, 0:T],
             lambda: ps2()[:, 0:T], lambda: ps2()[:, 0:T]],
        ]
        for ogi, og in enumerate(range(0, n_och, OGQ)):
            g = min(OGQ, n_och - og)
            views, keep = dense_group(ogi % 2)
            pss = views
            for ci in range(CT):
                wt = wsl_pool.tile([128, OG * 128], BF16, tag="wsl", name=named("t"))
                nc.sync.dma_start(
                    wt[:, :g * 128],
                    w_qkT[ci * 128:(ci + 1) * 128, og * 128:(og + g) * 128],
                )
                for j in range(g):
                    nc.tensor.matmul(
                        pss[j],
                        wt[:, j * 128:(j + 1) * 128],
                        ht[ci][:],
                        start=(ci == 0), stop=(ci == CT - 1),
                    )
                if ci % 4 == 3:
                    flush_rope(1)
            flush_rope(99)  # safety: CT may be < 16
            # fp32 x rides the scalar-engine HWDGE queue, spread across the
            # qk phase: never blocks the SP weight stream, never races ahead
            for ci in (2 * ogi, 2 * ogi + 1):
                nc.scalar.dma_start(
                    xt[ci][:], xT[ci * 128:(ci + 1) * 128, :])
            nxt = []
            for j in range(g):
                src = rsrc_pool.tile([128, T], BF16, tag="ropesrc", name=named("t"))
                nc.scalar.copy(src[:], pss[j])
                nxt.append((og + j, src, qk_mkps[ogi % 2][j]))
            rope_pend = nxt
        flush_rope()

        # ================= v-proj + attention software pipeline ===========
        # v-proj group g (4 heads, 256 v-features) runs interleaved with the
        # attention chains for group g-1. Emission is slot-based: 16 slots
        # per group (one per contraction tile ci); attention ops hang off
        # fixed slot offsets so their PSUM/semaphore waits are satisfied by
        # the time the PE reaches them.
        hpg = 4                  # heads per v-proj group
        DVW = hpg * D            # 256
        NG = C // DVW            # 8 groups
        NSLOT = 16

        v1 = [v_pool.tile([128, H * 65], BF16, tag="v1", name=named("t"))
              for _ in range(KT)]
        for tc_i in range(KT):
            ones_slots = v1[tc_i].rearrange("p (h e) -> p h e", e=65)[:, :, 64]
            nc.vector.memset(ones_slots, 1.0)

        ctxT = [ctx_pool.tile([128, T], BF16, tag="ctx", name=named("t"))
                for _ in range(CT)]

        slots = {}

        def addop(s, fn):
            slots.setdefault(s, []).append(fn)

        vp_tags = [["a0", "a1"], ["a2", "a3"]]

        # ---- v-proj emission closures
        def emit_vgroup(g):
            tags = vp_tags[g % 2]
            state = {}

            def start():
                state["ps"] = [ps1(tags[0]), ps1(tags[1])]
            addop(g * NSLOT, start)

            def step(ci):
                def fn():
                    wt = wsl_pool.tile([128, OG * 128], BF16, tag="wsl",
                                       name=named("t"))
                    nc.sync.dma_start(
                        wt[:, :DVW],
                        w_vT[ci * 128:(ci + 1) * 128, g * DVW:(g + 1) * DVW],
                    )
                    for tc_i in range(KT):
                        b = state["ps"][tc_i // 2]
                        co = (tc_i % 2) * DVW
                        # start=True clears has_written for the WHOLE bank;
                        # only the bank's very first matmul may set it. The
                        # second column-region's first write still lands via
                        # per-element overwrite-where-clear.
                        nc.tensor.matmul(
                            b[:, co:co + DVW],
                            ht[ci][:, tc_i * 128:(tc_i + 1) * 128],
                            wt[:, :DVW],
                            start=(ci == 0 and tc_i % 2 == 0),
                            stop=(ci == CT - 1),
                            skip_group_check=True,
                        )
                return fn
            for ci in range(CT):
                addop(g * NSLOT + ci, step(ci))

            def copies():
                for tc_i in range(KT):
                    b = state["ps"][tc_i // 2]
                    co = (tc_i % 2) * DVW
                    dst = v1[tc_i][:, g * hpg * 65:(g + 1) * hpg * 65].rearrange(
                        "p (h e) -> p h e", e=65)[:, :, 0:64]
                    srcap = b[:, co:co + DVW].rearrange(
                        "p (h e) -> p h e", e=64)
                    nc.vector.tensor_copy(dst, srcap)
            addop((g + 1) * NSLOT, copies)


        # ---- attention emission closures for one head group
        def emit_agroup(g, S):
            """Attention for heads 4g..4g+3, slots S.."""
            h0 = g * hpg
            # per-pair reciprocal rows live at partitions 0/64 (legal matmul
            # base partitions); each row holds both heads' denominators in
            # its two 512-column halves
            lnden = rsrc_pool.tile([128, 2 * T], F32, tag="lnden", bufs=2,
                                   name=named("t"))
            rec4 = rsrc_pool.tile([128, 2 * T], BF16, tag="rec4", bufs=2,
                                  name=named("t"))
            pair_state = [{}, {}]

            def sc_step(p, kt):
                def fn():
                    st = pair_state[p]
                    ha, hb = h0 + 2 * p, h0 + 2 * p + 1
                    ncols = T - kt * 128
                    sS = ps2()
                    for k, h in enumerate((ha, hb)):
                        q_t = qkrot[(h * D) // 128]
                        k_t = qkrot[(C + h * D) // 128]
                        ro = (h * D) % 128
                        nc.tensor.matmul(
                            sS[:, k * T:k * T + ncols],
                            k_t[ro:ro + D, kt * 128:(kt + 1) * 128],
                            q_t[ro:ro + D, kt * 128:],
                            start=True, stop=True,
                        )
                    e = e_pool.tile([128, 2 * T], BF16, tag="e",
                                    name=named("t"))
                    nc.scalar.activation(
                        e.rearrange("p (b c) -> p b c", b=2)[:, :, 0:ncols],
                        sS.rearrange("p (b c) -> p b c", b=2)[:, :, 0:ncols],
                        mybir.ActivationFunctionType.Exp, scale=scale,
                    )
                    e_diag = e.rearrange("p (b c) -> p b c", b=2)[:, :, 0:128]
                    nc.vector.tensor_mul(
                        e_diag, e_diag,
                        sb_tri.unsqueeze(1).broadcast_to([128, 2, 128]))
                    st.setdefault("e", []).append(e)
                return fn

            def av_step(p, kt):
                def fn():
                    st = pair_state[p]
                    ha, hb = h0 + 2 * p, h0 + 2 * p + 1
                    if kt == 0:
                        st["ctx"] = psC()
                    ncols = T - kt * 128
                    e = st["e"][kt]
                    for k, h in enumerate((ha, hb)):
                        nc.tensor.matmul(
                            st["ctx"][0:65, k * T + kt * 128:(k + 1) * T],
                            v1[kt][:, h * 65:(h + 1) * 65],
                            e[:, k * T:k * T + ncols],
                            start=(kt == 0), stop=(kt == KT - 1),
                            skip_group_check=True,
                        )
                return fn

            def ln_step(p):
                def fn():
                    st = pair_state[p]
                    # both heads' denominators sit in row 64 of the two
                    # adjacent ctx banks: one contiguous [1, 2T] scalar op
                    nc.scalar.activation(
                        lnden[64 * p:64 * p + 1, :], st["ctx"][64:65, :],
                        mybir.ActivationFunctionType.Ln,
                    )
                return fn

            def cs_step(p):
                def fn():
                    st = pair_state[p]
                    cs = tr_pool.tile([128, T], BF16, tag="trb1",
                                      name=named("t"))
                    nc.vector.tensor_copy(cs[0:D, :], st["ctx"][0:D, 0:T])
                    nc.vector.tensor_copy(cs[D:128, :], st["ctx"][0:D, T:2 * T])
                    st["cs"] = cs
                return fn

            def recexp():
                # full-tile exp: only rows 0/32/64/96 are meaningful, but
                # scalar cost depends on free-dim only, partitions are free
                nc.scalar.activation(
                    rec4[:, :], lnden[:, :],
                    mybir.ActivationFunctionType.Exp, scale=-1.0,
                )
            # rec broadcast + final ctx write for pair p
            def flush(p):
                def fn():
                    st = pair_state[p]
                    # rec broadcast draws from the score super-tile tag: its
                    # FIFO predecessor is always a score tile freed by an
                    # exp, never a PE op queued behind this one (deadlock).
                    ps_r = ps2()
                    for k in range(2):
                        nc.tensor.matmul(
                            ps_r[k * D:(k + 1) * D, 0:T],
                            ones_all[64 * p:64 * p + 1, 0:D],
                            rec4[64 * p:64 * p + 1, k * T:(k + 1) * T],
                            start=True, stop=True,
                            tile_position=(64 * p, k * D),
                        )
                    nc.vector.tensor_mul(
                        ctxT[g * 2 + p][:, :], st["cs"][:, :], ps_r[:, 0:T])
                return fn

            for p in range(2):
                B = S + 8 * p
                addop(B + 0, sc_step(p, 0))
                addop(B + 2, sc_step(p, 1))
                addop(B + 2, av_step(p, 0))
                addop(B + 4, sc_step(p, 2))
                addop(B + 4, av_step(p, 1))
                addop(B + 6, sc_step(p, 3))
                addop(B + 6, av_step(p, 2))
                addop(B + 8, av_step(p, 3))
                addop(B + 9, ln_step(p))
                addop(B + 9, cs_step(p))
            addop(S + 18, recexp)
            addop(S + 18, flush(0))
            addop(S + 19, flush(1))

        for g in range(NG):
            emit_vgroup(g)
        for g in range(NG):
            emit_agroup(g, (g + 1) * NSLOT)

        # ---- o-proj og-group 0 drips into the attention tail as PE filler
        oproj_ps = {}
        OGO = 4

        def oproj_start0():
            views, keep = dense_group(0)
            oproj_ps["v"] = views
            oproj_ps["keep"] = keep
        addop(NG * NSLOT + 2, oproj_start0)

        def oproj_step0(ci):
            def fn():
                wt = wsl_pool.tile([128, OG * 128], BF16, tag="wsl",
                                   name=named("t"))
                nc.sync.dma_start(
                    wt[:, :OGO * 128], w_oT[ci * 128:(ci + 1) * 128, 0:OGO * 128],
                )
                for j in range(OGO):
                    nc.tensor.matmul(
                        oproj_ps["v"][j],
                        wt[:, j * 128:(j + 1) * 128],
                        ctxT[ci][:],
                        start=(ci == 0), stop=(ci == CT - 1),
                        skip_group_check=True,
                    )
            return fn
        # ci14/15 must land at/after the last head group's flushes
        # (slots NG*16+18/19) so the PE never waits on a DVE op that is
        # queued behind it.
        for ci in range(CT):
            addop(NG * NSLOT + 4 + ci, oproj_step0(ci))

        # ---- flush the slot schedule
        for s in range(max(slots) + 1):
            for fn in slots.get(s, []):
                fn()

        # ---- o-proj (T layout) + residual -> x2T; sum-of-squares for
        # rmsnorm2 accumulates via transient psum partials + sbuf adds.
        x2t = [None] * CT
        ss2_sb = sm_pool.tile([1, T], F32, tag="ss2", name=named("t"))
        sq_pend = []

        def oproj_consume(og, views):
            """Residual add + square only: no PE ops, so the next og-group's
            matmuls aren't queued behind this chain."""
            sqs = []
            for j in range(OGO):
                x2 = xt_pool.tile([128, T], F32, tag="xt", name=named("t"))
                nc.vector.tensor_add(x2[:], xt[og + j][:], views[j])
                x2t[og + j] = x2
                sq2 = tr_pool.tile([128, T], BF16, tag="trb0", name=named("t"))
                nc.scalar.square(sq2[:], x2[:])
                sqs.append(sq2)
            sq_pend.append((og, sqs))

        def oproj_partial_flush(use_a):
            """Emitted one og-group late: the sum-of-squares matmuls then
            queue behind the next group's dense matmuls, by which time the
            squares are long done."""
            og, sqs = sq_pend.pop(0)
            ps_part = ps1("a0") if use_a else psC()
            for j, sq2 in enumerate(sqs):
                nc.tensor.matmul(
                    ps_part[0:1, 0:T], ones_col[:], sq2[:],
                    start=(j == 0), stop=(j == OGO - 1),
                    skip_group_check=True,
                )
            if og == 0:
                nc.vector.tensor_copy(ss2_sb[:], ps_part[0:1, 0:T])
            else:
                nc.vector.tensor_add(ss2_sb[:], ss2_sb[:], ps_part[0:1, 0:T])

        oproj_consume(0, oproj_ps["v"])
        for ogi, og in enumerate(range(OGO, CT, OGO)):
            # parity flipped vs the og0 drip (which used set A) so
            # consecutive og-groups never wait on each other's banks
            views, keep = dense_group((ogi + 1) % 2)
            for ci in range(CT):
                wt = wsl_pool.tile([128, OG * 128], BF16, tag="wsl", name=named("t"))
                nc.sync.dma_start(
                    wt[:, :OGO * 128],
                    w_oT[ci * 128:(ci + 1) * 128, og * 128:(og + OGO) * 128],
                )
                for j in range(OGO):
                    nc.tensor.matmul(
                        views[j],
                        wt[:, j * 128:(j + 1) * 128],
                        ctxT[ci][:],
                        start=(ci == 0), stop=(ci == CT - 1),
                    )
                if ci == 6:
                    oproj_partial_flush(ogi % 2 == 0)
            oproj_consume(og, views)
        oproj_partial_flush(True)

        # ---- rmsnorm 2
        h2t = rmsnorm(x2t, 1, "ht", ss_sb=ss2_sb)

        # ---- FFN up + swiglu -> actT (bf16, I rows)
        actT = [None] * ICH
        GG = min(4, ICH)  # gate chunks per group (paired with value chunks)
        for gg in range(0, ICH, GG):
            g = min(GG, ICH - gg)
            gviews, gkeep = dense_group(0)
            vviews, vkeep = dense_group(1)
            for ci in range(CT):
                wt = wsl_pool.tile([128, OG * 128], BF16, tag="wsl", name=named("t"))
                nc.sync.dma_start(
                    wt[:, :g * 128],
                    w_upT[ci * 128:(ci + 1) * 128, gg * 128:(gg + g) * 128],
                )
                nc.sync.dma_start(
                    wt[:, GG * 128:(GG + g) * 128],
                    w_upT[ci * 128:(ci + 1) * 128,
                          I + gg * 128:I + (gg + g) * 128],
                )
                for j in range(g):
                    nc.tensor.matmul(
                        gviews[j], wt[:, j * 128:(j + 1) * 128],
                        h2t[ci][:],
                        start=(ci == 0), stop=(ci == CT - 1),
                    )
                    nc.tensor.matmul(
                        vviews[j],
                        wt[:, (GG + j) * 128:(GG + j + 1) * 128],
                        h2t[ci][:],
                        start=(ci == 0), stop=(ci == CT - 1),
                    )
            for j in range(g):
                sg = tr_pool.tile([128, T], BF16, tag="trb1", name=named("t"))
                nc.scalar.activation(
                    sg[:], gviews[j],
                    mybir.ActivationFunctionType.Silu,
                )
                a = qk_pool.tile([128, T], BF16, tag="qk", name=named("t"))
                nc.vector.tensor_mul(a[:], sg[:], vviews[j])
                actT[gg + j] = a

        # ---- FFN down + residual -> outT. The last 4 output tiles run
        # as two groups of 2 so the final add+store tail is shorter.
        down_groups = [(0, 4), (4, 4), (8, 4), (12, 2), (14, 2)]
        dg_views = [None] * len(down_groups)
        for ogi, (og, g) in enumerate(down_groups):
            if g == 4:
                views, keep = dense_group(ogi % 2)
            elif ogi == 3:
                c = psC()
                views = [c[:, 0:T], c[:, T:2 * T]]
            else:
                s = ps2()
                views = [s[:, 0:T], s[:, T:2 * T]]
            for ii in range(ICH):
                wt = wsl_pool.tile([128, OG * 128], BF16, tag="wsl", name=named("t"))
                nc.sync.dma_start(
                    wt[:, :g * 128],
                    w_downT[ii * 128:(ii + 1) * 128, og * 128:(og + g) * 128],
                )
                for j in range(g):
                    nc.tensor.matmul(
                        views[j],
                        wt[:, j * 128:(j + 1) * 128],
                        actT[ii][:],
                        start=(ii == 0), stop=(ii == ICH - 1),
                    )
            for j in range(g):
                o_sb = tr_pool.tile([128, T], F32, tag="trf", bufs=2, name=named("t"))
                nc.vector.tensor_add(o_sb[:], x2t[og + j][:], views[j])
                # output rides the gpsimd software DMA queue: never blocks
                # the down-proj weight stream on the SP queue
                nc.gpsimd.dma_start(
                    outT[(og + j) * 128:(og + j + 1) * 128, :], o_sb[:],
                )

    _split_excess_waits(nc)
    return nc


def make_core_inputs(cfg: Cfg, x_shard, w_qkv, w_o, w_up, w_down,
                     attn_norm_w, ffn_norm_w, pos0, shared):
    """Host-side prep of one core's input map. x_shard [T, C] fp32.
    `shared` caches the (identical) weight arrays across cores."""
    T, C, D = cfg.T, cfg.C, cfg.D
    if not shared:
        shared["w_qkT"] = np.ascontiguousarray(w_qkv[:2 * C].T).astype(NPBF16)
        shared["w_vT"] = np.ascontiguousarray(
            w_qkv[2 * C:3 * C].T).astype(NPBF16)
        shared["w_oT"] = np.ascontiguousarray(w_o.T).astype(NPBF16)
        shared["w_upT"] = np.ascontiguousarray(w_up.T).astype(NPBF16)
        shared["w_downT"] = np.ascontiguousarray(w_down.T).astype(NPBF16)
        shared["nw1"] = attn_norm_w.reshape(1, C).astype(NPBF16)
        shared["nw2"] = ffn_norm_w.reshape(1, C).astype(NPBF16)
        k_idx = np.arange(128)
        shared["trimask"] = (
            k_idx[:, None] <= k_idx[None, :]).astype(NPBF16)
        psw = np.zeros((128, 128), dtype=NPBF16)
        psw[k_idx ^ 32, k_idx] = 1.0  # lhsT[j, p] = 1 iff j == p ^ 32
        shared["pswap"] = psw
    inv = (1.0 / ROPE_THETA ** (np.arange(0, D, 2) / D)).astype(np.float64)
    pos = np.arange(pos0, pos0 + T, dtype=np.float64)
    fr = np.outer(pos, inv)                       # [T, D/2]
    emb = np.concatenate([fr, fr], axis=-1)       # [T, D]
    cosT = np.cos(emb).T.astype(np.float32)       # [D, T]
    sinT = np.sin(emb).T.astype(np.float32)
    nsinT = sinT.copy()
    nsinT[:D // 2] *= -1.0
    reps = 128 // D
    nsin2 = np.tile(nsinT, (reps, 1))
    perm = np.arange(128) ^ 32
    s2 = nsin2[perm]          # s2[p] = nsin2[p ^ 32]
    xt_host = np.ascontiguousarray(x_shard.T)
    return {
        "xT": xt_host.astype(np.float32),
        "xbT": xt_host.astype(NPBF16),
        "cosT2": np.tile(cosT, (reps, 1)).astype(NPBF16),
        "nsinT2": s2.astype(NPBF16),
        **shared,
    }


def kernel(x, attn_norm_w, ffn_norm_w, w_qkv, w_o, w_up, w_down,
           _trace=False, _tmpdir=None):
    x = np.asarray(x, dtype=np.float32)
    attn_norm_w = np.asarray(attn_norm_w, dtype=np.float32)
    ffn_norm_w = np.asarray(ffn_norm_w, dtype=np.float32)
    w_qkv = np.asarray(w_qkv, dtype=np.float32)
    w_o = np.asarray(w_o, dtype=np.float32)
    w_up = np.asarray(w_up, dtype=np.float32)
    w_down = np.asarray(w_down, dtype=np.float32)

    B, S, C = x.shape
    cfg = Cfg(T=512, C=C, H=C // 64, D=64, I=2 * C)
    n_blocks = S // cfg.T
    assert B * n_blocks == 8

    nc = build_program(cfg)

    shared = {}
    in_maps = []
    for core in range(8):
        b, blk = divmod(core, n_blocks)
        sl = slice(blk * cfg.T, (blk + 1) * cfg.T)
        in_maps.append(make_core_inputs(
            cfg, x[b, sl], w_qkv, w_o, w_up, w_down,
            attn_norm_w, ffn_norm_w, pos0=blk * cfg.T, shared=shared,
        ))

    res = run_bass_kernel_spmd(
        nc, in_maps, core_ids=list(range(8)),
        trace=_trace, tmpdir=_tmpdir,
    )

    out = np.empty((B, S, C), dtype=np.float32)
    for core in range(8):
        b, blk = divmod(core, n_blocks)
        sl = slice(blk * cfg.T, (blk + 1) * cfg.T)
        out[b, sl] = res.results[core]["outT"].T
    kernel.last_result = res
    return out


# revision 34
# speedup vs baseline: 1.0180x; 1.0180x over previous
"""Trainium2 Bass kernel for nn_BlockAttentionResidual.

Transformer block: RMSNorm -> QKV -> RoPE -> block-diagonal causal attention
(4 blocks of 512) -> o-proj + residual -> RMSNorm -> SwiGLU FFN + residual.
Shapes: x [2, 2048, 2048], 32 heads x 64, inter 4096.

Sharding: 8 cores = (batch 2) x (4 sequence blocks of 512 tokens). The
attention mask is block-diagonal causal with block size 512, so each core's
512-token slice is fully independent -> no collectives.

On-device layout is "T layout" [feature, token] throughout, because every
matmul contracts the feature dim, which must sit on SBUF partitions for the
PE. Matmuls run in bf16 with fp32 PSUM accumulation; softmax skips the max
subtraction (scores are small); the causal mask is only needed on the
128x128 diagonal chunks of each 512 block; softmax denominators come from a
ones-column appended to V; per-token broadcasts across partitions are done
as rank-1 outer-product matmuls on the PE.

v2: the v-projection and attention are software-pipelined: v-proj for head
group g runs interleaved (one emission "slot" at a time) with the attention
chains for head group g-1, so the PE chews dense v-proj matmuls while the
scalar engine runs the softmax exps that the attention matmuls wait on.
PSUM's 8 banks are hand-allocated via single-buffer pool tags. Softmax exps
for a head pair run as one scalar op over a 2-bank PSUM super-tile; the
softmax reciprocal runs as per-head Ln (doubling as the cross-partition
collect) + one batched Exp per 4-head group.
"""

import math
from contextlib import ExitStack

import ml_dtypes
import numpy as np

import concourse.bass as bass
import concourse.mybir as mybir
import concourse.tile as tile
from concourse.bass_utils import run_bass_kernel_spmd
from concourse.vector_clock import ScopedClock

F32 = mybir.dt.float32
BF16 = mybir.dt.bfloat16
NPBF16 = ml_dtypes.bfloat16

EPS = 1e-5
ROPE_THETA = 10000.0


# --- workaround: this walrus build allows only one sem wait per CTRL-queue
# instruction (Drain/NoOp), but Tile's tail drain aggregates every
# outstanding wait onto a single SP Drain. Spread them over SP NOPs.
def _patched_drain_and_barrier(self, tick_clock, wait_clock):
    nop_inst = self.nc.sync.nop(nofuse=True)
    wait_clock.add_sem_waits(
        nop_inst.ins, ScopedClock({None: tick_clock.global_clock})
    )
    si = nop_inst.ins.sync_info
    waits = list(si.on_wait) if si is not None else []
    if len(waits) > 1:
        si.on_wait = waits[:1]
        for w in waits[1:]:
            n2 = self.nc.sync.nop(nofuse=True)
            if n2.ins.sync_info is None:
                n2.ins.sync_info = mybir.SyncInfo(on_wait=[w], on_update=[])
            else:
                n2.ins.sync_info.on_wait = [w]
    self.nc.sync.drain()
    self.nc.all_engine_barrier()
    assert self.sems is not None
    popped = self.nc._tile_sem_poison_stack.pop()
    assert popped is self._sem_poison
    self.nc.clear_and_free_semaphores(list(self.sems.allocated().values()))
    self.nc.all_engine_barrier()


tile.TileContext._drain_and_barrier = _patched_drain_and_barrier


def _split_excess_waits(nc, maxw=1):
    """This walrus build rejects instructions carrying more than one sync
    wait; hoist extras onto single-wait NOPs queued just before on the same
    engine."""
    fn = nc.m.functions[0]
    for bb in fn.blocks:
        out = []
        changed = False
        for inst in bb.instructions:
            si = getattr(inst, "sync_info", None)
            waits = list(si.on_wait) if si is not None else []
            if len(waits) > maxw:
                changed = True
                for w in waits[:-maxw]:
                    nop = mybir.InstNoOp(
                        name=nc.get_next_instruction_name(), ins=[], outs=[])
                    nop.engine = inst.engine
                    nop.sync_info = mybir.SyncInfo(on_wait=[w], on_update=[])
                    out.append(nop)
                si.on_wait = waits[-maxw:]
            out.append(inst)
        if changed:
            bb.instructions = out


class Cfg:
    def __init__(self, T=512, C=2048, H=32, D=64, I=4096):
        self.T = T          # tokens per core (one attention block)
        self.C = C          # hidden
        self.H = H          # heads
        self.D = D          # head dim (must be 64)
        self.I = I          # ffn inner
        assert D == 64 and C == H * D
        assert T % 128 == 0 and C % 128 == 0 and I % 128 == 0


def build_program(cfg: Cfg):
    T, C, H, D, I = cfg.T, cfg.C, cfg.H, cfg.D, cfg.I
    CT = C // 128            # hidden tiles
    KT = T // 128            # token chunks (and attention k-chunks)
    QKN = 2 * C              # q+k feature rows
    ICH = I // 128           # ffn inner chunks
    scale = 1.0 / math.sqrt(D)
    OG = 8                   # psum group width for dense matmul phases

    nc = bass.Bass("TRN2", target_bir_lowering=False, debug=False)

    xT = nc.dram_tensor("xT", (C, T), F32, kind="ExternalInput").ap()
    xbT = nc.dram_tensor("xbT", (C, T), BF16, kind="ExternalInput").ap()
    w_qkT = nc.dram_tensor("w_qkT", (C, QKN), BF16, kind="ExternalInput").ap()
    w_vT = nc.dram_tensor("w_vT", (C, C), BF16, kind="ExternalInput").ap()
    w_oT = nc.dram_tensor("w_oT", (C, C), BF16, kind="ExternalInput").ap()
    w_upT = nc.dram_tensor("w_upT", (C, 2 * I), BF16, kind="ExternalInput").ap()
    w_downT = nc.dram_tensor("w_downT", (I, C), BF16, kind="ExternalInput").ap()
    nw1 = nc.dram_tensor("nw1", (1, C), BF16, kind="ExternalInput").ap()
    nw2 = nc.dram_tensor("nw2", (1, C), BF16, kind="ExternalInput").ap()
    cosT2 = nc.dram_tensor("cosT2", (128, T), BF16, kind="ExternalInput").ap()
    nsinT2 = nc.dram_tensor("nsinT2", (128, T), BF16, kind="ExternalInput").ap()
    trimask = nc.dram_tensor("trimask", (128, 128), BF16, kind="ExternalInput").ap()
    pswap = nc.dram_tensor("pswap", (128, 128), BF16, kind="ExternalInput").ap()
    outT = nc.dram_tensor("outT", (C, T), F32, kind="ExternalOutput").ap()

    with tile.TileContext(nc) as tc, ExitStack() as ctx:
        consts = ctx.enter_context(tc.tile_pool(name="consts", bufs=1))
        # xt slots also serve x2 (x dies at the o-proj residual add); +2
        # rolling slots so the x2 alloc never waits on its own free.
        xt_pool = ctx.enter_context(tc.tile_pool(name="xt", bufs=CT + 2))
        ht_pool = ctx.enter_context(tc.tile_pool(name="ht", bufs=CT))
        qk_pool = ctx.enter_context(
            tc.tile_pool(name="qk", bufs=max(2 * CT, ICH)))
        v_pool = ctx.enter_context(tc.tile_pool(name="v", bufs=KT))
        ctx_pool = ctx.enter_context(tc.tile_pool(name="ctx", bufs=CT))
        wsl_pool = ctx.enter_context(tc.tile_pool(name="wsl", bufs=8))
        tr_pool = ctx.enter_context(tc.tile_pool(name="tr", bufs=4))
        e_pool = ctx.enter_context(tc.tile_pool(name="e", bufs=4))
        sm_pool = ctx.enter_context(tc.tile_pool(name="sm", bufs=2))
        rsrc_pool = ctx.enter_context(tc.tile_pool(name="rsrc", bufs=9))
        # PSUM: 8 banks, hand-allocated. tags a0..a3/c0/c1 are one bank
        # ([128,512] f32); tag S is a two-bank super-tile ([128,1024]).
        ps_pool = ctx.enter_context(
            tc.tile_pool(name="ps", bufs=1, space="PSUM"))

        _nm = [0]

        def named(base):
            _nm[0] += 1
            return f"{base}{_nm[0]}"

        def ps1(tag):
            return ps_pool.tile([128, T], F32, tag=tag, bufs=1,
                                name=named("ps"))

        def ps2():
            return ps_pool.tile([128, 2 * T], F32, tag="S", bufs=1,
                                name=named("psS"))

        def psC():
            return ps_pool.tile([128, 2 * T], F32, tag="C", bufs=1,
                                name=named("psC"))

        SET_A = ["a0", "a1", "a2", "a3"]

        def dense_group(parity):
            """4 psum [128,T] views for a dense og-group: even parity uses
            banks a0..a3, odd uses the S super-tile halves + c0/c1. Returns
            (views, tiles-to-keep-alive)."""
            if parity == 0:
                ts = [ps1(t) for t in SET_A]
                return [t[:, :] for t in ts], ts
            s = ps2()
            c = psC()
            # C halves first: at phase junctions they free earlier than S,
            # letting the next group's first matmuls start sooner
            return [c[:, 0:T], c[:, T:2 * T], s[:, 0:T], s[:, T:2 * T]], [s, c]

        # ---- constants
        sb_cos = consts.tile([128, T], BF16)
        nc.sync.dma_start(sb_cos[:], cosT2[:])
        sb_nsin = consts.tile([128, T], BF16)
        nc.sync.dma_start(sb_nsin[:], nsinT2[:])
        sb_tri = consts.tile([128, 128], BF16)
        nc.sync.dma_start(sb_tri[:], trimask[:])
        sb_psw = consts.tile([128, 128], BF16)
        nc.sync.dma_start(sb_psw[:], pswap[:])
        sb_nw = consts.tile([1, 2 * C], BF16)
        nc.sync.dma_start(sb_nw[0:1, 0:C], nw1[:])
        nc.sync.dma_start(sb_nw[0:1, C:2 * C], nw2[:])
        ones_col = consts.tile([128, 1], BF16)
        nc.vector.memset(ones_col[:], 1.0)
        ones_all = consts.tile([128, D], BF16)
        nc.vector.memset(ones_all[:], 1.0)
        eps_t = consts.tile([1, 1], F32)
        nc.vector.memset(eps_t[:], EPS)

        # ---- load a bf16 copy of x for rmsnorm1 (half the critical-path
        # DMA bytes); the fp32 x, needed only for the o-proj residual,
        # streams in later spread across the attention-phase slots. xb
        # tiles ride the qk pool rotation ahead of the qkrot tiles.
        xb = []
        for ci in range(CT):
            t = qk_pool.tile([128, T], BF16, tag="qk", name=named("t"))
            nc.sync.dma_start(t[:], xbT[ci * 128:(ci + 1) * 128, :])
            xb.append(t)
        xt = [xt_pool.tile([128, T], F32, tag="xt", name=named("t"))
              for _ in range(CT)]

        def rmsnorm(x_tiles, nw_row, out_tag, ss_sb=None):
            """x_tiles: CT fp32 [128, T] tiles (T layout) -> bf16 tiles of
            x * nw[c] * rstd[t]. ss_sb: optionally an sbuf [1, T] tile
            already holding the sum of squares."""
            if ss_sb is None:
                ps_ss = psC()
                for ci in range(CT):
                    sq = tr_pool.tile([128, T], BF16, tag="trb0", name=named("t"))
                    nc.scalar.square(sq[:], x_tiles[ci][:])
                    nc.tensor.matmul(
                        ps_ss[0:1, 0:T], ones_col[:], sq[:],
                        start=(ci == 0), stop=(ci == CT - 1),
                    )
                ss_in = ps_ss[0:1, 0:T]
            else:
                ss_in = ss_sb[0:1, :]
            s_sb = sm_pool.tile([1, T], F32, tag="s1", name=named("t"))
            nc.scalar.activation(
                s_sb[:], ss_in, mybir.ActivationFunctionType.Ln,
                bias=eps_t[:], scale=1.0 / C,
            )
            rstd = sm_pool.tile([1, T], BF16, tag="s2", name=named("t"))
            nc.scalar.activation(
                rstd[:], s_sb[:], mybir.ActivationFunctionType.Exp,
                scale=-0.5,
            )
            out = []
            cyc = ["a0", "a1", "a2", "a3"]
            for ci in range(CT):
                ps_sc = ps1(cyc[ci % len(cyc)])
                nc.tensor.matmul(
                    ps_sc[:, :],
                    sb_nw[0:1, nw_row * C + ci * 128:nw_row * C + (ci + 1) * 128],
                    rstd[:],
                    start=True, stop=True,
                )
                h = ht_pool.tile([128, T], BF16, tag=out_tag, name=named("t"))
                nc.vector.tensor_mul(h[:], x_tiles[ci][:], ps_sc[:, :])
                out.append(h)
            return out

        # ---- rmsnorm 1
        ht = rmsnorm(xb, 0, "ht")

        # ---- q/k projection (T layout) + RoPE
        qkrot = [None] * (QKN // 128)
        n_och = QKN // 128
        OGQ = 4
        rope_pend = []

        def flush_rope(n=99):
            # emitted one og-group late (dripped between matmul bursts) so
            # the swap matmul's inputs are ready and the DVE work is spread
            for _ in range(min(n, len(rope_pend))):
                idx, src, mk_ps = rope_pend.pop(0)
                a = tr_pool.tile([128, T], BF16, tag="trb2", bufs=2, name=named("t"))
                nc.vector.tensor_mul(a[:], src[:], sb_cos[:])
                m = tr_pool.tile([128, T], BF16, tag="trb3", bufs=2, name=named("t"))
                nc.vector.tensor_mul(m[:], src[:], sb_nsin[:])
                ps_b = mk_ps()
                nc.tensor.matmul(ps_b, sb_psw[:], m[:],
                                 start=True, stop=True)
                rot = qk_pool.tile([128, T], BF16, tag="qk", name=named("t"))
                nc.vector.tensor_add(rot[:], a[:], ps_b)
                qkrot[idx] = rot

        qk_mkps = [
            [lambda t=t: ps1(t)[:, :] for t in SET_A],
            [lambda: psC()[:, 0:T], lambda: psC()[:, 0:T],
             lambda: ps2()[:, 0:T], lambda: ps2()[:, 0:T]],
        ]
        for ogi, og in enumerate(range(0, n_och, OGQ)):
            g = min(OGQ, n_och - og)
            views, keep = dense_group(ogi % 2)
            pss = views
            for ci in range(CT):
                wt = wsl_pool.tile([128, OG * 128], BF16, tag="wsl", name=named("t"))
                nc.sync.dma_start(
                    wt[:, :g * 128],
                    w_qkT[ci * 128:(ci + 1) * 128, og * 128:(og + g) * 128],
                )
                for j in range(g):
                    nc.tensor.matmul(
                        pss[j],
                        wt[:, j * 128:(j + 1) * 128],
                        ht[ci][:],
                        start=(ci == 0), stop=(ci == CT - 1),
                    )
                if ci % 4 == 3:
                    flush_rope(1)
            flush_rope(99)  # safety: CT may be < 16
            nxt = []
            for j in range(g):
                src = rsrc_pool.tile([128, T], BF16, tag="ropesrc", name=named("t"))
                nc.scalar.copy(src[:], pss[j])
                nxt.append((og + j, src, qk_mkps[ogi % 2][j]))
            rope_pend = nxt
        flush_rope()

        # fp32 x rides the gpsimd software DMA queue (never blocks the SP
        # weight stream); each transfer is pinned behind a qkrot tile via a
        # tiny WAW-dependency copy so the scheduler can't hoist it into the
        # ramp where it would steal HBM bandwidth from xb/weights.
        for ci in range(CT):
            nc.vector.tensor_copy(xt[ci][0:1, 0:1], qkrot[2 * ci][0:1, 0:1])
            nc.gpsimd.dma_start(xt[ci][:], xT[ci * 128:(ci + 1) * 128, :])

        # ================= v-proj + attention software pipeline ===========
        # v-proj group g (4 heads, 256 v-features) runs interleaved with the
        # attention chains for group g-1. Emission is slot-based: 16 slots
        # per group (one per contraction tile ci); attention ops hang off
        # fixed slot offsets so their PSUM/semaphore waits are satisfied by
        # the time the PE reaches them.
        hpg = 4                  # heads per v-proj group
        DVW = hpg * D            # 256
        NG = C // DVW            # 8 groups
        NSLOT = 16

        v1 = [v_pool.tile([128, H * 65], BF16, tag="v1", name=named("t"))
              for _ in range(KT)]
        for tc_i in range(KT):
            ones_slots = v1[tc_i].rearrange("p (h e) -> p h e", e=65)[:, :, 64]
            nc.vector.memset(ones_slots, 1.0)

        ctxT = [ctx_pool.tile([128, T], BF16, tag="ctx", name=named("t"))
                for _ in range(CT)]

        slots = {}

        def addop(s, fn):
            slots.setdefault(s, []).append(fn)

        vp_tags = [["a0", "a1"], ["a2", "a3"]]

        # ---- v-proj emission closures
        def emit_vgroup(g):
            tags = vp_tags[g % 2]
            state = {}

            def start():
                state["ps"] = [ps1(tags[0]), ps1(tags[1])]
            addop(g * NSLOT, start)

            def step(ci):
                def fn():
                    wt = wsl_pool.tile([128, OG * 128], BF16, tag="wsl",
                                       name=named("t"))
                    nc.sync.dma_start(
                        wt[:, :DVW],
                        w_vT[ci * 128:(ci + 1) * 128, g * DVW:(g + 1) * DVW],
                    )
                    for tc_i in range(KT):
                        b = state["ps"][tc_i // 2]
                        co = (tc_i % 2) * DVW
                        # start=True clears has_written for the WHOLE bank;
                        # only the bank's very first matmul may set it. The
                        # second column-region's first write still lands via
                        # per-element overwrite-where-clear.
                        nc.tensor.matmul(
                            b[:, co:co + DVW],
                            ht[ci][:, tc_i * 128:(tc_i + 1) * 128],
                            wt[:, :DVW],
                            start=(ci == 0 and tc_i % 2 == 0),
                            stop=(ci == CT - 1),
                            skip_group_check=True,
                        )
                return fn
            for ci in range(CT):
                addop(g * NSLOT + ci, step(ci))

            def copies():
                for tc_i in range(KT):
                    b = state["ps"][tc_i // 2]
                    co = (tc_i % 2) * DVW
                    dst = v1[tc_i][:, g * hpg * 65:(g + 1) * hpg * 65].rearrange(
                        "p (h e) -> p h e", e=65)[:, :, 0:64]
                    srcap = b[:, co:co + DVW].rearrange(
                        "p (h e) -> p h e", e=64)
                    nc.vector.tensor_copy(dst, srcap)
            addop((g + 1) * NSLOT, copies)


        # ---- attention emission closures for one head group
        def emit_agroup(g, S):
            """Attention for heads 4g..4g+3, slots S.."""
            h0 = g * hpg
            # per-pair reciprocal rows live at partitions 0/64 (legal matmul
            # base partitions); each row holds both heads' denominators in
            # its two 512-column halves
            lnden = rsrc_pool.tile([128, 2 * T], F32, tag="lnden", bufs=2,
                                   name=named("t"))
            rec4 = rsrc_pool.tile([128, 2 * T], BF16, tag="rec4", bufs=2,
                                  name=named("t"))
            pair_state = [{}, {}]

            def sc_step(p, kt):
                def fn():
                    st = pair_state[p]
                    ha, hb = h0 + 2 * p, h0 + 2 * p + 1
                    ncols = T - kt * 128
                    sS = ps2()
                    for k, h in enumerate((ha, hb)):
                        q_t = qkrot[(h * D) // 128]
                        k_t = qkrot[(C + h * D) // 128]
                        ro = (h * D) % 128
                        nc.tensor.matmul(
                            sS[:, k * T:k * T + ncols],
                            k_t[ro:ro + D, kt * 128:(kt + 1) * 128],
                            q_t[ro:ro + D, kt * 128:],
                            start=True, stop=True,
                        )
                    e = e_pool.tile([128, 2 * T], BF16, tag="e",
                                    name=named("t"))
                    nc.scalar.activation(
                        e.rearrange("p (b c) -> p b c", b=2)[:, :, 0:ncols],
                        sS.rearrange("p (b c) -> p b c", b=2)[:, :, 0:ncols],
                        mybir.ActivationFunctionType.Exp, scale=scale,
                    )
                    e_diag = e.rearrange("p (b c) -> p b c", b=2)[:, :, 0:128]
                    nc.vector.tensor_mul(
                        e_diag, e_diag,
                        sb_tri.unsqueeze(1).broadcast_to([128, 2, 128]))
                    st.setdefault("e", []).append(e)
                return fn

            def av_step(p, kt):
                def fn():
                    st = pair_state[p]
                    ha, hb = h0 + 2 * p, h0 + 2 * p + 1
                    if kt == 0:
                        st["ctx"] = psC()
                    ncols = T - kt * 128
                    e = st["e"][kt]
                    for k, h in enumerate((ha, hb)):
                        nc.tensor.matmul(
                            st["ctx"][0:65, k * T + kt * 128:(k + 1) * T],
                            v1[kt][:, h * 65:(h + 1) * 65],
                            e[:, k * T:k * T + ncols],
                            start=(kt == 0), stop=(kt == KT - 1),
                            skip_group_check=True,
                        )
                return fn

            def ln_step(p):
                def fn():
                    st = pair_state[p]
                    # both heads' denominators sit in row 64 of the two
                    # adjacent ctx banks: one contiguous [1, 2T] scalar op
                    nc.scalar.activation(
                        lnden[64 * p:64 * p + 1, :], st["ctx"][64:65, :],
                        mybir.ActivationFunctionType.Ln,
                    )
                return fn

            def cs_step(p):
                def fn():
                    st = pair_state[p]
                    cs = tr_pool.tile([128, T], BF16, tag="trb1",
                                      name=named("t"))
                    nc.vector.tensor_copy(cs[0:D, :], st["ctx"][0:D, 0:T])
                    nc.vector.tensor_copy(cs[D:128, :], st["ctx"][0:D, T:2 * T])
                    st["cs"] = cs
                return fn

            def recexp():
                # full-tile exp: only rows 0/32/64/96 are meaningful, but
                # scalar cost depends on free-dim only, partitions are free
                nc.scalar.activation(
                    rec4[:, :], lnden[:, :],
                    mybir.ActivationFunctionType.Exp, scale=-1.0,
                )
            # rec broadcast + final ctx write for pair p
            def flush(p):
                def fn():
                    st = pair_state[p]
                    # rec broadcast draws from the score super-tile tag: its
                    # FIFO predecessor is always a score tile freed by an
                    # exp, never a PE op queued behind this one (deadlock).
                    ps_r = ps2()
                    for k in range(2):
                        nc.tensor.matmul(
                            ps_r[k * D:(k + 1) * D, 0:T],
                            ones_all[64 * p:64 * p + 1, 0:D],
                            rec4[64 * p:64 * p + 1, k * T:(k + 1) * T],
                            start=True, stop=True,
                            tile_position=(64 * p, k * D),
                        )
                    nc.vector.tensor_mul(
                        ctxT[g * 2 + p][:, :], st["cs"][:, :], ps_r[:, 0:T])
                return fn

            for p in range(2):
                B = S + 8 * p
                addop(B + 0, sc_step(p, 0))
                addop(B + 2, sc_step(p, 1))
                addop(B + 2, av_step(p, 0))
                addop(B + 4, sc_step(p, 2))
                addop(B + 4, av_step(p, 1))
                addop(B + 6, sc_step(p, 3))
                addop(B + 6, av_step(p, 2))
                addop(B + 8, av_step(p, 3))
                addop(B + 9, ln_step(p))
                addop(B + 9, cs_step(p))
            addop(S + 18, recexp)
            addop(S + 18, flush(0))
            addop(S + 19, flush(1))

        for g in range(NG):
            emit_vgroup(g)
        for g in range(NG):
            emit_agroup(g, (g + 1) * NSLOT)

        # ---- o-proj og-group 0 drips into the attention tail as PE filler
        oproj_ps = {}
        OGO = 4

        def oproj_start0():
            views, keep = dense_group(0)
            oproj_ps["v"] = views
            oproj_ps["keep"] = keep
        addop(NG * NSLOT + 2, oproj_start0)

        def oproj_step0(ci):
            def fn():
                wt = wsl_pool.tile([128, OG * 128], BF16, tag="wsl",
                                   name=named("t"))
                nc.sync.dma_start(
                    wt[:, :OGO * 128], w_oT[ci * 128:(ci + 1) * 128, 0:OGO * 128],
                )
                for j in range(OGO):
                    nc.tensor.matmul(
                        oproj_ps["v"][j],
                        wt[:, j * 128:(j + 1) * 128],
                        ctxT[ci][:],
                        start=(ci == 0), stop=(ci == CT - 1),
                        skip_group_check=True,
                    )
            return fn
        # ci14/15 must land at/after the last head group's flushes
        # (slots NG*16+18/19) so the PE never waits on a DVE op that is
        # queued behind it.
        for ci in range(CT):
            addop(NG * NSLOT + 4 + ci, oproj_step0(ci))

        # ---- flush the slot schedule
        for s in range(max(slots) + 1):
            for fn in slots.get(s, []):
                fn()

        # ---- o-proj (T layout) + residual -> x2T; sum-of-squares for
        # rmsnorm2 accumulates via transient psum partials + sbuf adds.
        x2t = [None] * CT
        ss2_sb = sm_pool.tile([1, T], F32, tag="ss2", name=named("t"))
        sq_pend = []

        def oproj_consume(og, views):
            """Residual add + square only: no PE ops, so the next og-group's
            matmuls aren't queued behind this chain."""
            sqs = []
            for j in range(OGO):
                x2 = xt_pool.tile([128, T], F32, tag="xt", name=named("t"))
                nc.vector.tensor_add(x2[:], xt[og + j][:], views[j])
                x2t[og + j] = x2
                sq2 = tr_pool.tile([128, T], BF16, tag="trb0", name=named("t"))
                nc.scalar.square(sq2[:], x2[:])
                sqs.append(sq2)
            sq_pend.append((og, sqs))

        def oproj_partial_flush(use_a):
            """Emitted one og-group late: the sum-of-squares matmuls then
            queue behind the next group's dense matmuls, by which time the
            squares are long done."""
            og, sqs = sq_pend.pop(0)
            ps_part = ps1("a0") if use_a else psC()
            for j, sq2 in enumerate(sqs):
                nc.tensor.matmul(
                    ps_part[0:1, 0:T], ones_col[:], sq2[:],
                    start=(j == 0), stop=(j == OGO - 1),
                    skip_group_check=True,
                )
            if og == 0:
                nc.vector.tensor_copy(ss2_sb[:], ps_part[0:1, 0:T])
            else:
                nc.vector.tensor_add(ss2_sb[:], ss2_sb[:], ps_part[0:1, 0:T])

        oproj_consume(0, oproj_ps["v"])
        for ogi, og in enumerate(range(OGO, CT, OGO)):
            # parity flipped vs the og0 drip (which used set A) so
            # consecutive og-groups never wait on each other's banks
            views, keep = dense_group((ogi + 1) % 2)
            for ci in range(CT):
                wt = wsl_pool.tile([128, OG * 128], BF16, tag="wsl", name=named("t"))
                nc.sync.dma_start(
                    wt[:, :OGO * 128],
                    w_oT[ci * 128:(ci + 1) * 128, og * 128:(og + OGO) * 128],
                )
                for j in range(OGO):
                    nc.tensor.matmul(
                        views[j],
                        wt[:, j * 128:(j + 1) * 128],
                        ctxT[ci][:],
                        start=(ci == 0), stop=(ci == CT - 1),
                    )
                if ci == 6:
                    oproj_partial_flush(ogi % 2 == 0)
            oproj_consume(og, views)
        oproj_partial_flush(True)

        # ---- rmsnorm 2
        h2t = rmsnorm(x2t, 1, "ht", ss_sb=ss2_sb)

        # ---- FFN up + swiglu -> actT (bf16, I rows)
        actT = [None] * ICH
        GG = min(4, ICH)  # gate chunks per group (paired with value chunks)
        for gg in range(0, ICH, GG):
            g = min(GG, ICH - gg)
            gviews, gkeep = dense_group(0)
            vviews, vkeep = dense_group(1)
            for ci in range(CT):
                wt = wsl_pool.tile([128, OG * 128], BF16, tag="wsl", name=named("t"))
                nc.sync.dma_start(
                    wt[:, :g * 128],
                    w_upT[ci * 128:(ci + 1) * 128, gg * 128:(gg + g) * 128],
                )
                nc.sync.dma_start(
                    wt[:, GG * 128:(GG + g) * 128],
                    w_upT[ci * 128:(ci + 1) * 128,
                          I + gg * 128:I + (gg + g) * 128],
                )
                for j in range(g):
                    nc.tensor.matmul(
                        gviews[j], wt[:, j * 128:(j + 1) * 128],
                        h2t[ci][:],
                        start=(ci == 0), stop=(ci == CT - 1),
                    )
                    nc.tensor.matmul(
                        vviews[j],
                        wt[:, (GG + j) * 128:(GG + j + 1) * 128],
                        h2t[ci][:],
                        start=(ci == 0), stop=(ci == CT - 1),
                    )
            for j in range(g):
                sg = tr_pool.tile([128, T], BF16, tag="trb1", name=named("t"))
                nc.scalar.activation(
                    sg[:], gviews[j],
                    mybir.ActivationFunctionType.Silu,
                )
                a = qk_pool.tile([128, T], BF16, tag="qk", name=named("t"))
                nc.vector.tensor_mul(a[:], sg[:], vviews[j])
                actT[gg + j] = a

        # ---- FFN down + residual -> outT
        down_groups = [(0, 4), (4, 4), (8, 4), (12, 4)]
        for ogi, (og, g) in enumerate(down_groups):
            views, keep = dense_group(ogi % 2)
            for ii in range(ICH):
                wt = wsl_pool.tile([128, OG * 128], BF16, tag="wsl", name=named("t"))
                nc.sync.dma_start(
                    wt[:, :g * 128],
                    w_downT[ii * 128:(ii + 1) * 128, og * 128:(og + g) * 128],
                )
                for j in range(g):
                    nc.tensor.matmul(
                        views[j],
                        wt[:, j * 128:(j + 1) * 128],
                        actT[ii][:],
                        start=(ii == 0), stop=(ii == ICH - 1),
                    )
            for j in range(g):
                o_sb = tr_pool.tile([128, T], F32, tag="trf", bufs=2, name=named("t"))
                nc.vector.tensor_add(o_sb[:], x2t[og + j][:], views[j])
                # output rides the gpsimd software DMA queue: never blocks
                # the down-proj weight stream on the SP queue
                nc.gpsimd.dma_start(
                    outT[(og + j) * 128:(og + j + 1) * 128, :], o_sb[:],
                )

    _split_excess_waits(nc)
    return nc


def make_core_inputs(cfg: Cfg, x_shard, w_qkv, w_o, w_up, w_down,
                     attn_norm_w, ffn_norm_w, pos0, shared):
    """Host-side prep of one core's input map. x_shard [T, C] fp32.
    `shared` caches the (identical) weight arrays across cores."""
    T, C, D = cfg.T, cfg.C, cfg.D
    if not shared:
        shared["w_qkT"] = np.ascontiguousarray(w_qkv[:2 * C].T).astype(NPBF16)
        shared["w_vT"] = np.ascontiguousarray(
            w_qkv[2 * C:3 * C].T).astype(NPBF16)
        shared["w_oT"] = np.ascontiguousarray(w_o.T).astype(NPBF16)
        shared["w_upT"] = np.ascontiguousarray(w_up.T).astype(NPBF16)
        shared["w_downT"] = np.ascontiguousarray(w_down.T).astype(NPBF16)
        shared["nw1"] = attn_norm_w.reshape(1, C).astype(NPBF16)
        shared["nw2"] = ffn_norm_w.reshape(1, C).astype(NPBF16)
        k_idx = np.arange(128)
        shared["trimask"] = (
            k_idx[:, None] <= k_idx[None, :]).astype(NPBF16)
        psw = np.zeros((128, 128), dtype=NPBF16)
        psw[k_idx ^ 32, k_idx] = 1.0  # lhsT[j, p] = 1 iff j == p ^ 32
        shared["pswap"] = psw
    inv = (1.0 / ROPE_THETA ** (np.arange(0, D, 2) / D)).astype(np.float64)
    pos = np.arange(pos0, pos0 + T, dtype=np.float64)
    fr = np.outer(pos, inv)                       # [T, D/2]
    emb = np.concatenate([fr, fr], axis=-1)       # [T, D]
    cosT = np.cos(emb).T.astype(np.float32)       # [D, T]
    sinT = np.sin(emb).T.astype(np.float32)
    nsinT = sinT.copy()
    nsinT[:D // 2] *= -1.0
    reps = 128 // D
    nsin2 = np.tile(nsinT, (reps, 1))
    perm = np.arange(128) ^ 32
    s2 = nsin2[perm]          # s2[p] = nsin2[p ^ 32]
    xt_host = np.ascontiguousarray(x_shard.T)
    return {
        "xT": xt_host.astype(np.float32),
        "xbT": xt_host.astype(NPBF16),
        "cosT2": np.tile(cosT, (reps, 1)).astype(NPBF16),
        "nsinT2": s2.astype(NPBF16),
        **shared,
    }


def kernel(x, attn_norm_w, ffn_norm_w, w_qkv, w_o, w_up, w_down,
           _trace=False, _tmpdir=None):
    x = np.asarray(x, dtype=np.float32)
    attn_norm_w = np.asarray(attn_norm_w, dtype=np.float32)
    ffn_norm_w = np.asarray(ffn_norm_w, dtype=np.float32)
    w_qkv = np.asarray(w_qkv, dtype=np.float32)
    w_o = np.asarray(w_o, dtype=np.float32)
    w_up = np.asarray(w_up, dtype=np.float32)
    w_down = np.asarray(w_down, dtype=np.float32)

    B, S, C = x.shape
    cfg = Cfg(T=512, C=C, H=C // 64, D=64, I=2 * C)
    n_blocks = S // cfg.T
    assert B * n_blocks == 8

    nc = build_program(cfg)

    shared = {}
    in_maps = []
    for core in range(8):
        b, blk = divmod(core, n_blocks)
        sl = slice(blk * cfg.T, (blk + 1) * cfg.T)
        in_maps.append(make_core_inputs(
            cfg, x[b, sl], w_qkv, w_o, w_up, w_down,
            attn_norm_w, ffn_norm_w, pos0=blk * cfg.T, shared=shared,
        ))

    res = run_bass_kernel_spmd(
        nc, in_maps, core_ids=list(range(8)),
        trace=_trace, tmpdir=_tmpdir,
    )

    out = np.empty((B, S, C), dtype=np.float32)
    for core in range(8):
        b, blk = divmod(core, n_blocks)
        sl = slice(blk * cfg.T, (blk + 1) * cfg.T)
        out[b, sl] = res.results[core]["outT"].T
    kernel.last_result = res
    return out


# revision 35
# speedup vs baseline: 1.0243x; 1.0061x over previous
"""Trainium2 Bass kernel for nn_BlockAttentionResidual.

Transformer block: RMSNorm -> QKV -> RoPE -> block-diagonal causal attention
(4 blocks of 512) -> o-proj + residual -> RMSNorm -> SwiGLU FFN + residual.
Shapes: x [2, 2048, 2048], 32 heads x 64, inter 4096.

Sharding: 8 cores = (batch 2) x (4 sequence blocks of 512 tokens). The
attention mask is block-diagonal causal with block size 512, so each core's
512-token slice is fully independent -> no collectives.

On-device layout is "T layout" [feature, token] throughout, because every
matmul contracts the feature dim, which must sit on SBUF partitions for the
PE. Matmuls run in bf16 with fp32 PSUM accumulation; softmax skips the max
subtraction (scores are small); the causal mask is only needed on the
128x128 diagonal chunks of each 512 block; softmax denominators come from a
ones-column appended to V; per-token broadcasts across partitions are done
as rank-1 outer-product matmuls on the PE.

v2: the v-projection and attention are software-pipelined: v-proj for head
group g runs interleaved (one emission "slot" at a time) with the attention
chains for head group g-1, so the PE chews dense v-proj matmuls while the
scalar engine runs the softmax exps that the attention matmuls wait on.
PSUM's 8 banks are hand-allocated via single-buffer pool tags. Softmax exps
for a head pair run as one scalar op over a 2-bank PSUM super-tile; the
softmax reciprocal runs as per-head Ln (doubling as the cross-partition
collect) + one batched Exp per 4-head group.
"""

import math
from contextlib import ExitStack

import ml_dtypes
import numpy as np

import concourse.bass as bass
import concourse.mybir as mybir
import concourse.tile as tile
from concourse.bass_utils import run_bass_kernel_spmd
from concourse.vector_clock import ScopedClock

F32 = mybir.dt.float32
BF16 = mybir.dt.bfloat16
NPBF16 = ml_dtypes.bfloat16

EPS = 1e-5
ROPE_THETA = 10000.0


# --- workaround: this walrus build allows only one sem wait per CTRL-queue
# instruction (Drain/NoOp), but Tile's tail drain aggregates every
# outstanding wait onto a single SP Drain. Spread them over SP NOPs.
def _patched_drain_and_barrier(self, tick_clock, wait_clock):
    nop_inst = self.nc.sync.nop(nofuse=True)
    wait_clock.add_sem_waits(
        nop_inst.ins, ScopedClock({None: tick_clock.global_clock})
    )
    si = nop_inst.ins.sync_info
    waits = list(si.on_wait) if si is not None else []
    if len(waits) > 1:
        si.on_wait = waits[:1]
        for w in waits[1:]:
            n2 = self.nc.sync.nop(nofuse=True)
            if n2.ins.sync_info is None:
                n2.ins.sync_info = mybir.SyncInfo(on_wait=[w], on_update=[])
            else:
                n2.ins.sync_info.on_wait = [w]
    self.nc.sync.drain()
    self.nc.all_engine_barrier()
    assert self.sems is not None
    popped = self.nc._tile_sem_poison_stack.pop()
    assert popped is self._sem_poison
    self.nc.clear_and_free_semaphores(list(self.sems.allocated().values()))
    self.nc.all_engine_barrier()


tile.TileContext._drain_and_barrier = _patched_drain_and_barrier


def _split_excess_waits(nc, maxw=1):
    """This walrus build rejects instructions carrying more than one sync
    wait; hoist extras onto single-wait NOPs queued just before on the same
    engine."""
    fn = nc.m.functions[0]
    for bb in fn.blocks:
        out = []
        changed = False
        for inst in bb.instructions:
            si = getattr(inst, "sync_info", None)
            waits = list(si.on_wait) if si is not None else []
            if len(waits) > maxw:
                changed = True
                for w in waits[:-maxw]:
                    nop = mybir.InstNoOp(
                        name=nc.get_next_instruction_name(), ins=[], outs=[])
                    nop.engine = inst.engine
                    nop.sync_info = mybir.SyncInfo(on_wait=[w], on_update=[])
                    out.append(nop)
                si.on_wait = waits[-maxw:]
            out.append(inst)
        if changed:
            bb.instructions = out


class Cfg:
    def __init__(self, T=512, C=2048, H=32, D=64, I=4096):
        self.T = T          # tokens per core (one attention block)
        self.C = C          # hidden
        self.H = H          # heads
        self.D = D          # head dim (must be 64)
        self.I = I          # ffn inner
        assert D == 64 and C == H * D
        assert T % 128 == 0 and C % 128 == 0 and I % 128 == 0


def build_program(cfg: Cfg):
    T, C, H, D, I = cfg.T, cfg.C, cfg.H, cfg.D, cfg.I
    CT = C // 128            # hidden tiles
    KT = T // 128            # token chunks (and attention k-chunks)
    QKN = 2 * C              # q+k feature rows
    ICH = I // 128           # ffn inner chunks
    scale = 1.0 / math.sqrt(D)
    OG = 8                   # psum group width for dense matmul phases

    nc = bass.Bass("TRN2", target_bir_lowering=False, debug=False)

    xT = nc.dram_tensor("xT", (C, T), F32, kind="ExternalInput").ap()
    xbT = nc.dram_tensor("xbT", (C, T), BF16, kind="ExternalInput").ap()
    w_qkT = nc.dram_tensor("w_qkT", (C, QKN), BF16, kind="ExternalInput").ap()
    w_vT = nc.dram_tensor("w_vT", (C, C), BF16, kind="ExternalInput").ap()
    w_oT = nc.dram_tensor("w_oT", (C, C), BF16, kind="ExternalInput").ap()
    w_upT = nc.dram_tensor("w_upT", (C, 2 * I), BF16, kind="ExternalInput").ap()
    w_downT = nc.dram_tensor("w_downT", (I, C), BF16, kind="ExternalInput").ap()
    nw1 = nc.dram_tensor("nw1", (1, C), BF16, kind="ExternalInput").ap()
    nw2 = nc.dram_tensor("nw2", (1, C), BF16, kind="ExternalInput").ap()
    cosT2 = nc.dram_tensor("cosT2", (128, T), BF16, kind="ExternalInput").ap()
    nsinT2 = nc.dram_tensor("nsinT2", (128, T), BF16, kind="ExternalInput").ap()
    trimask = nc.dram_tensor("trimask", (128, 128), BF16, kind="ExternalInput").ap()
    pswap = nc.dram_tensor("pswap", (128, 128), BF16, kind="ExternalInput").ap()
    outT = nc.dram_tensor("outT", (C, T), F32, kind="ExternalOutput").ap()

    with tile.TileContext(nc) as tc, ExitStack() as ctx:
        consts = ctx.enter_context(tc.tile_pool(name="consts", bufs=1))
        # xt slots also serve x2 (x dies at the o-proj residual add); +2
        # rolling slots so the x2 alloc never waits on its own free.
        xt_pool = ctx.enter_context(tc.tile_pool(name="xt", bufs=CT + 2))
        ht_pool = ctx.enter_context(tc.tile_pool(name="ht", bufs=CT))
        qk_pool = ctx.enter_context(
            tc.tile_pool(name="qk", bufs=max(2 * CT, ICH)))
        v_pool = ctx.enter_context(tc.tile_pool(name="v", bufs=KT))
        ctx_pool = ctx.enter_context(tc.tile_pool(name="ctx", bufs=CT))
        wsl_pool = ctx.enter_context(tc.tile_pool(name="wsl", bufs=8))
        tr_pool = ctx.enter_context(tc.tile_pool(name="tr", bufs=4))
        e_pool = ctx.enter_context(tc.tile_pool(name="e", bufs=4))
        sm_pool = ctx.enter_context(tc.tile_pool(name="sm", bufs=2))
        rsrc_pool = ctx.enter_context(tc.tile_pool(name="rsrc", bufs=9))
        # PSUM: 8 banks, hand-allocated. tags a0..a3/c0/c1 are one bank
        # ([128,512] f32); tag S is a two-bank super-tile ([128,1024]).
        ps_pool = ctx.enter_context(
            tc.tile_pool(name="ps", bufs=1, space="PSUM"))

        _nm = [0]

        def named(base):
            _nm[0] += 1
            return f"{base}{_nm[0]}"

        def ps1(tag):
            return ps_pool.tile([128, T], F32, tag=tag, bufs=1,
                                name=named("ps"))

        def ps2():
            return ps_pool.tile([128, 2 * T], F32, tag="S", bufs=1,
                                name=named("psS"))

        def psC():
            return ps_pool.tile([128, 2 * T], F32, tag="C", bufs=1,
                                name=named("psC"))

        SET_A = ["a0", "a1", "a2", "a3"]

        def dense_group(parity):
            """4 psum [128,T] views for a dense og-group: even parity uses
            banks a0..a3, odd uses the S super-tile halves + c0/c1. Returns
            (views, tiles-to-keep-alive)."""
            if parity == 0:
                ts = [ps1(t) for t in SET_A]
                return [t[:, :] for t in ts], ts
            s = ps2()
            c = psC()
            # C halves first: at phase junctions they free earlier than S,
            # letting the next group's first matmuls start sooner
            return [c[:, 0:T], c[:, T:2 * T], s[:, 0:T], s[:, T:2 * T]], [s, c]

        # ---- constants
        sb_cos = consts.tile([128, T], BF16)
        nc.sync.dma_start(sb_cos[:], cosT2[:])
        sb_nsin = consts.tile([128, T], BF16)
        nc.sync.dma_start(sb_nsin[:], nsinT2[:])
        sb_tri = consts.tile([128, 128], BF16)
        nc.sync.dma_start(sb_tri[:], trimask[:])
        sb_psw = consts.tile([128, 128], BF16)
        nc.sync.dma_start(sb_psw[:], pswap[:])
        sb_nw = consts.tile([1, 2 * C], BF16)
        nc.sync.dma_start(sb_nw[0:1, 0:C], nw1[:])
        nc.sync.dma_start(sb_nw[0:1, C:2 * C], nw2[:])
        ones_col = consts.tile([128, 1], BF16)
        nc.vector.memset(ones_col[:], 1.0)
        ones_all = consts.tile([128, D], BF16)
        nc.vector.memset(ones_all[:], 1.0)
        eps_t = consts.tile([1, 1], F32)
        nc.vector.memset(eps_t[:], EPS)

        # ---- load a bf16 copy of x for rmsnorm1 (half the critical-path
        # DMA bytes); the fp32 x, needed only for the o-proj residual,
        # streams in later spread across the attention-phase slots. xb
        # tiles ride the qk pool rotation ahead of the qkrot tiles.
        xb = []
        for ci in range(CT):
            t = qk_pool.tile([128, T], BF16, tag="qk", name=named("t"))
            nc.sync.dma_start(t[:], xbT[ci * 128:(ci + 1) * 128, :])
            xb.append(t)
        xt = [xt_pool.tile([128, T], F32, tag="xt", name=named("t"))
              for _ in range(CT)]

        def rmsnorm(x_tiles, nw_row, out_tag, ss_sb=None):
            """x_tiles: CT fp32 [128, T] tiles (T layout) -> bf16 tiles of
            x * nw[c] * rstd[t]. ss_sb: optionally an sbuf [1, T] tile
            already holding the sum of squares."""
            if ss_sb is None:
                ps_ss = psC()
                for ci in range(CT):
                    sq = tr_pool.tile([128, T], BF16, tag="trb0", name=named("t"))
                    nc.scalar.square(sq[:], x_tiles[ci][:])
                    nc.tensor.matmul(
                        ps_ss[0:1, 0:T], ones_col[:], sq[:],
                        start=(ci == 0), stop=(ci == CT - 1),
                    )
                ss_in = ps_ss[0:1, 0:T]
            else:
                ss_in = ss_sb[0:1, :]
            s_sb = sm_pool.tile([1, T], F32, tag="s1", name=named("t"))
            nc.scalar.activation(
                s_sb[:], ss_in, mybir.ActivationFunctionType.Ln,
                bias=eps_t[:], scale=1.0 / C,
            )
            rstd = sm_pool.tile([1, T], BF16, tag="s2", name=named("t"))
            nc.scalar.activation(
                rstd[:], s_sb[:], mybir.ActivationFunctionType.Exp,
                scale=-0.5,
            )
            out = []
            cyc = ["a0", "a1", "a2", "a3"]
            for ci in range(CT):
                ps_sc = ps1(cyc[ci % len(cyc)])
                nc.tensor.matmul(
                    ps_sc[:, :],
                    sb_nw[0:1, nw_row * C + ci * 128:nw_row * C + (ci + 1) * 128],
                    rstd[:],
                    start=True, stop=True,
                )
                h = ht_pool.tile([128, T], BF16, tag=out_tag, name=named("t"))
                nc.vector.tensor_mul(h[:], x_tiles[ci][:], ps_sc[:, :])
                out.append(h)
            return out

        # ---- rmsnorm 1
        ht = rmsnorm(xb, 0, "ht")

        # ---- q/k projection (T layout) + RoPE
        qkrot = [None] * (QKN // 128)
        n_och = QKN // 128
        OGQ = 4
        rope_pend = []

        def flush_rope(n=99):
            # emitted one og-group late (dripped between matmul bursts) so
            # the swap matmul's inputs are ready and the DVE work is spread
            for _ in range(min(n, len(rope_pend))):
                idx, src, mk_ps = rope_pend.pop(0)
                a = tr_pool.tile([128, T], BF16, tag="trb2", bufs=2, name=named("t"))
                nc.vector.tensor_mul(a[:], src[:], sb_cos[:])
                m = tr_pool.tile([128, T], BF16, tag="trb3", bufs=2, name=named("t"))
                nc.vector.tensor_mul(m[:], src[:], sb_nsin[:])
                ps_b = mk_ps()
                nc.tensor.matmul(ps_b, sb_psw[:], m[:],
                                 start=True, stop=True)
                rot = qk_pool.tile([128, T], BF16, tag="qk", name=named("t"))
                nc.vector.tensor_add(rot[:], a[:], ps_b)
                qkrot[idx] = rot

        qk_mkps = [
            [lambda t=t: ps1(t)[:, :] for t in SET_A],
            [lambda: psC()[:, 0:T], lambda: psC()[:, 0:T],
             lambda: ps2()[:, 0:T], lambda: ps2()[:, 0:T]],
        ]
        for ogi, og in enumerate(range(0, n_och, OGQ)):
            g = min(OGQ, n_och - og)
            views, keep = dense_group(ogi % 2)
            pss = views
            for ci in range(CT):
                wt = wsl_pool.tile([128, OG * 128], BF16, tag="wsl", name=named("t"))
                nc.sync.dma_start(
                    wt[:, :g * 128],
                    w_qkT[ci * 128:(ci + 1) * 128, og * 128:(og + g) * 128],
                )
                for j in range(g):
                    nc.tensor.matmul(
                        pss[j],
                        wt[:, j * 128:(j + 1) * 128],
                        ht[ci][:],
                        start=(ci == 0), stop=(ci == CT - 1),
                    )
                if ci % 4 == 3:
                    flush_rope(1)
            flush_rope(99)  # safety: CT may be < 16
            nxt = []
            for j in range(g):
                src = rsrc_pool.tile([128, T], BF16, tag="ropesrc", name=named("t"))
                nc.scalar.copy(src[:], pss[j])
                nxt.append((og + j, src, qk_mkps[ogi % 2][j]))
            rope_pend = nxt
        flush_rope()

        # fp32 x rides the gpsimd software DMA queue (never blocks the SP
        # weight stream); each transfer is pinned behind a qkrot tile via a
        # tiny WAW-dependency copy so the scheduler can't hoist it into the
        # ramp where it would steal HBM bandwidth from xb/weights.
        for ci in range(CT):
            nc.vector.tensor_copy(xt[ci][0:1, 0:1], qkrot[ci][0:1, 0:1])
            nc.gpsimd.dma_start(xt[ci][:], xT[ci * 128:(ci + 1) * 128, :])

        # ================= v-proj + attention software pipeline ===========
        # v-proj group g (4 heads, 256 v-features) runs interleaved with the
        # attention chains for group g-1. Emission is slot-based: 16 slots
        # per group (one per contraction tile ci); attention ops hang off
        # fixed slot offsets so their PSUM/semaphore waits are satisfied by
        # the time the PE reaches them.
        hpg = 4                  # heads per v-proj group
        DVW = hpg * D            # 256
        NG = C // DVW            # 8 groups
        NSLOT = 16

        v1 = [v_pool.tile([128, H * 65], BF16, tag="v1", name=named("t"))
              for _ in range(KT)]
        for tc_i in range(KT):
            ones_slots = v1[tc_i].rearrange("p (h e) -> p h e", e=65)[:, :, 64]
            nc.vector.memset(ones_slots, 1.0)

        ctxT = [ctx_pool.tile([128, T], BF16, tag="ctx", name=named("t"))
                for _ in range(CT)]

        slots = {}

        def addop(s, fn):
            slots.setdefault(s, []).append(fn)

        vp_tags = [["a0", "a1"], ["a2", "a3"]]

        # ---- v-proj emission closures
        def emit_vgroup(g):
            tags = vp_tags[g % 2]
            state = {}

            def start():
                state["ps"] = [ps1(tags[0]), ps1(tags[1])]
            addop(g * NSLOT, start)

            def step(ci):
                def fn():
                    wt = wsl_pool.tile([128, OG * 128], BF16, tag="wsl",
                                       name=named("t"))
                    nc.sync.dma_start(
                        wt[:, :DVW],
                        w_vT[ci * 128:(ci + 1) * 128, g * DVW:(g + 1) * DVW],
                    )
                    for tc_i in range(KT):
                        b = state["ps"][tc_i // 2]
                        co = (tc_i % 2) * DVW
                        # start=True clears has_written for the WHOLE bank;
                        # only the bank's very first matmul may set it. The
                        # second column-region's first write still lands via
                        # per-element overwrite-where-clear.
                        nc.tensor.matmul(
                            b[:, co:co + DVW],
                            ht[ci][:, tc_i * 128:(tc_i + 1) * 128],
                            wt[:, :DVW],
                            start=(ci == 0 and tc_i % 2 == 0),
                            stop=(ci == CT - 1),
                            skip_group_check=True,
                        )
                return fn
            for ci in range(CT):
                addop(g * NSLOT + ci, step(ci))

            def copies():
                for tc_i in range(KT):
                    b = state["ps"][tc_i // 2]
                    co = (tc_i % 2) * DVW
                    dst = v1[tc_i][:, g * hpg * 65:(g + 1) * hpg * 65].rearrange(
                        "p (h e) -> p h e", e=65)[:, :, 0:64]
                    srcap = b[:, co:co + DVW].rearrange(
                        "p (h e) -> p h e", e=64)
                    nc.vector.tensor_copy(dst, srcap)
            addop((g + 1) * NSLOT, copies)


        # ---- attention emission closures for one head group
        def emit_agroup(g, S):
            """Attention for heads 4g..4g+3, slots S.."""
            h0 = g * hpg
            # per-pair reciprocal rows live at partitions 0/64 (legal matmul
            # base partitions); each row holds both heads' denominators in
            # its two 512-column halves
            lnden = rsrc_pool.tile([128, 2 * T], F32, tag="lnden", bufs=2,
                                   name=named("t"))
            rec4 = rsrc_pool.tile([128, 2 * T], BF16, tag="rec4", bufs=2,
                                  name=named("t"))
            pair_state = [{}, {}]

            def sc_step(p, kt):
                def fn():
                    st = pair_state[p]
                    ha, hb = h0 + 2 * p, h0 + 2 * p + 1
                    ncols = T - kt * 128
                    sS = ps2()
                    for k, h in enumerate((ha, hb)):
                        q_t = qkrot[(h * D) // 128]
                        k_t = qkrot[(C + h * D) // 128]
                        ro = (h * D) % 128
                        nc.tensor.matmul(
                            sS[:, k * T:k * T + ncols],
                            k_t[ro:ro + D, kt * 128:(kt + 1) * 128],
                            q_t[ro:ro + D, kt * 128:],
                            start=True, stop=True,
                        )
                    e = e_pool.tile([128, 2 * T], BF16, tag="e",
                                    name=named("t"))
                    nc.scalar.activation(
                        e.rearrange("p (b c) -> p b c", b=2)[:, :, 0:ncols],
                        sS.rearrange("p (b c) -> p b c", b=2)[:, :, 0:ncols],
                        mybir.ActivationFunctionType.Exp, scale=scale,
                    )
                    e_diag = e.rearrange("p (b c) -> p b c", b=2)[:, :, 0:128]
                    nc.vector.tensor_mul(
                        e_diag, e_diag,
                        sb_tri.unsqueeze(1).broadcast_to([128, 2, 128]))
                    st.setdefault("e", []).append(e)
                return fn

            def av_step(p, kt):
                def fn():
                    st = pair_state[p]
                    ha, hb = h0 + 2 * p, h0 + 2 * p + 1
                    if kt == 0:
                        st["ctx"] = psC()
                    ncols = T - kt * 128
                    e = st["e"][kt]
                    for k, h in enumerate((ha, hb)):
                        nc.tensor.matmul(
                            st["ctx"][0:65, k * T + kt * 128:(k + 1) * T],
                            v1[kt][:, h * 65:(h + 1) * 65],
                            e[:, k * T:k * T + ncols],
                            start=(kt == 0), stop=(kt == KT - 1),
                            skip_group_check=True,
                        )
                return fn

            def ln_step(p):
                def fn():
                    st = pair_state[p]
                    # both heads' denominators sit in row 64 of the two
                    # adjacent ctx banks: one contiguous [1, 2T] scalar op
                    nc.scalar.activation(
                        lnden[64 * p:64 * p + 1, :], st["ctx"][64:65, :],
                        mybir.ActivationFunctionType.Ln,
                    )
                return fn

            def cs_step(p):
                def fn():
                    st = pair_state[p]
                    cs = tr_pool.tile([128, T], BF16, tag="trb1",
                                      name=named("t"))
                    nc.vector.tensor_copy(cs[0:D, :], st["ctx"][0:D, 0:T])
                    nc.vector.tensor_copy(cs[D:128, :], st["ctx"][0:D, T:2 * T])
                    st["cs"] = cs
                return fn

            def recexp():
                # full-tile exp: only rows 0/32/64/96 are meaningful, but
                # scalar cost depends on free-dim only, partitions are free
                nc.scalar.activation(
                    rec4[:, :], lnden[:, :],
                    mybir.ActivationFunctionType.Exp, scale=-1.0,
                )
            # rec broadcast + final ctx write for pair p
            def flush(p):
                def fn():
                    st = pair_state[p]
                    # rec broadcast draws from the score super-tile tag: its
                    # FIFO predecessor is always a score tile freed by an
                    # exp, never a PE op queued behind this one (deadlock).
                    ps_r = ps2()
                    for k in range(2):
                        nc.tensor.matmul(
                            ps_r[k * D:(k + 1) * D, 0:T],
                            ones_all[64 * p:64 * p + 1, 0:D],
                            rec4[64 * p:64 * p + 1, k * T:(k + 1) * T],
                            start=True, stop=True,
                            tile_position=(64 * p, k * D),
                        )
                    nc.vector.tensor_mul(
                        ctxT[g * 2 + p][:, :], st["cs"][:, :], ps_r[:, 0:T])
                return fn

            for p in range(2):
                B = S + 8 * p
                addop(B + 0, sc_step(p, 0))
                addop(B + 2, sc_step(p, 1))
                addop(B + 2, av_step(p, 0))
                addop(B + 4, sc_step(p, 2))
                addop(B + 4, av_step(p, 1))
                addop(B + 6, sc_step(p, 3))
                addop(B + 6, av_step(p, 2))
                addop(B + 8, av_step(p, 3))
                addop(B + 9, ln_step(p))
                addop(B + 9, cs_step(p))
            addop(S + 18, recexp)
            addop(S + 18, flush(0))
            addop(S + 19, flush(1))

        for g in range(NG):
            emit_vgroup(g)
        for g in range(NG):
            emit_agroup(g, (g + 1) * NSLOT)

        # ---- o-proj og-group 0 drips into the attention tail as PE filler
        oproj_ps = {}
        OGO = 4

        def oproj_start0():
            views, keep = dense_group(0)
            oproj_ps["v"] = views
            oproj_ps["keep"] = keep
        addop(NG * NSLOT + 2, oproj_start0)

        def oproj_step0(ci):
            def fn():
                wt = wsl_pool.tile([128, OG * 128], BF16, tag="wsl",
                                   name=named("t"))
                nc.sync.dma_start(
                    wt[:, :OGO * 128], w_oT[ci * 128:(ci + 1) * 128, 0:OGO * 128],
                )
                for j in range(OGO):
                    nc.tensor.matmul(
                        oproj_ps["v"][j],
                        wt[:, j * 128:(j + 1) * 128],
                        ctxT[ci][:],
                        start=(ci == 0), stop=(ci == CT - 1),
                        skip_group_check=True,
                    )
            return fn
        # ci14/15 must land at/after the last head group's flushes
        # (slots NG*16+18/19) so the PE never waits on a DVE op that is
        # queued behind it.
        for ci in range(CT):
            addop(NG * NSLOT + 4 + ci, oproj_step0(ci))

        # ---- flush the slot schedule
        for s in range(max(slots) + 1):
            for fn in slots.get(s, []):
                fn()

        # ---- o-proj (T layout) + residual -> x2T; sum-of-squares for
        # rmsnorm2 accumulates via transient psum partials + sbuf adds.
        x2t = [None] * CT
        ss2_sb = sm_pool.tile([1, T], F32, tag="ss2", name=named("t"))
        sq_pend = []

        def oproj_consume(og, views):
            """Residual add + square only: no PE ops, so the next og-group's
            matmuls aren't queued behind this chain."""
            sqs = []
            for j in range(OGO):
                x2 = xt_pool.tile([128, T], F32, tag="xt", name=named("t"))
                nc.vector.tensor_add(x2[:], xt[og + j][:], views[j])
                x2t[og + j] = x2
                sq2 = tr_pool.tile([128, T], BF16, tag="trb0", name=named("t"))
                nc.scalar.square(sq2[:], x2[:])
                sqs.append(sq2)
            sq_pend.append((og, sqs))

        def oproj_partial_flush(use_a):
            """Emitted one og-group late: the sum-of-squares matmuls then
            queue behind the next group's dense matmuls, by which time the
            squares are long done."""
            og, sqs = sq_pend.pop(0)
            ps_part = ps1("a0") if use_a else psC()
            for j, sq2 in enumerate(sqs):
                nc.tensor.matmul(
                    ps_part[0:1, 0:T], ones_col[:], sq2[:],
                    start=(j == 0), stop=(j == OGO - 1),
                    skip_group_check=True,
                )
            if og == 0:
                nc.vector.tensor_copy(ss2_sb[:], ps_part[0:1, 0:T])
            else:
                nc.vector.tensor_add(ss2_sb[:], ss2_sb[:], ps_part[0:1, 0:T])

        oproj_consume(0, oproj_ps["v"])
        for ogi, og in enumerate(range(OGO, CT, OGO)):
            # parity flipped vs the og0 drip (which used set A) so
            # consecutive og-groups never wait on each other's banks
            views, keep = dense_group((ogi + 1) % 2)
            for ci in range(CT):
                wt = wsl_pool.tile([128, OG * 128], BF16, tag="wsl", name=named("t"))
                nc.sync.dma_start(
                    wt[:, :OGO * 128],
                    w_oT[ci * 128:(ci + 1) * 128, og * 128:(og + OGO) * 128],
                )
                for j in range(OGO):
                    nc.tensor.matmul(
                        views[j],
                        wt[:, j * 128:(j + 1) * 128],
                        ctxT[ci][:],
                        start=(ci == 0), stop=(ci == CT - 1),
                    )
                if ci == 6:
                    oproj_partial_flush(ogi % 2 == 0)
            oproj_consume(og, views)
        oproj_partial_flush(True)

        # ---- rmsnorm 2
        h2t = rmsnorm(x2t, 1, "ht", ss_sb=ss2_sb)

        # ---- FFN up + swiglu -> actT (bf16, I rows)
        actT = [None] * ICH
        GG = min(4, ICH)  # gate chunks per group (paired with value chunks)
        for gg in range(0, ICH, GG):
            g = min(GG, ICH - gg)
            gviews, gkeep = dense_group(0)
            vviews, vkeep = dense_group(1)
            for ci in range(CT):
                wt = wsl_pool.tile([128, OG * 128], BF16, tag="wsl", name=named("t"))
                nc.sync.dma_start(
                    wt[:, :g * 128],
                    w_upT[ci * 128:(ci + 1) * 128, gg * 128:(gg + g) * 128],
                )
                nc.sync.dma_start(
                    wt[:, GG * 128:(GG + g) * 128],
                    w_upT[ci * 128:(ci + 1) * 128,
                          I + gg * 128:I + (gg + g) * 128],
                )
                for j in range(g):
                    nc.tensor.matmul(
                        gviews[j], wt[:, j * 128:(j + 1) * 128],
                        h2t[ci][:],
                        start=(ci == 0), stop=(ci == CT - 1),
                    )
                    nc.tensor.matmul(
                        vviews[j],
                        wt[:, (GG + j) * 128:(GG + j + 1) * 128],
                        h2t[ci][:],
                        start=(ci == 0), stop=(ci == CT - 1),
                    )
            for j in range(g):
                sg = tr_pool.tile([128, T], BF16, tag="trb1", name=named("t"))
                nc.scalar.activation(
                    sg[:], gviews[j],
                    mybir.ActivationFunctionType.Silu,
                )
                a = qk_pool.tile([128, T], BF16, tag="qk", name=named("t"))
                nc.vector.tensor_mul(a[:], sg[:], vviews[j])
                actT[gg + j] = a

        # ---- FFN down + residual -> outT
        down_groups = [(0, 4), (4, 4), (8, 4), (12, 4)]
        for ogi, (og, g) in enumerate(down_groups):
            views, keep = dense_group(ogi % 2)
            for ii in range(ICH):
                wt = wsl_pool.tile([128, OG * 128], BF16, tag="wsl", name=named("t"))
                nc.sync.dma_start(
                    wt[:, :g * 128],
                    w_downT[ii * 128:(ii + 1) * 128, og * 128:(og + g) * 128],
                )
                for j in range(g):
                    nc.tensor.matmul(
                        views[j],
                        wt[:, j * 128:(j + 1) * 128],
                        actT[ii][:],
                        start=(ii == 0), stop=(ii == ICH - 1),
                    )
            for j in range(g):
                o_sb = tr_pool.tile([128, T], F32, tag="trf", bufs=2, name=named("t"))
                nc.vector.tensor_add(o_sb[:], x2t[og + j][:], views[j])
                # output rides the gpsimd software DMA queue: never blocks
                # the down-proj weight stream on the SP queue
                nc.gpsimd.dma_start(
                    outT[(og + j) * 128:(og + j + 1) * 128, :], o_sb[:],
                )

    _split_excess_waits(nc)
    return nc


def make_core_inputs(cfg: Cfg, x_shard, w_qkv, w_o, w_up, w_down,
                     attn_norm_w, ffn_norm_w, pos0, shared):
    """Host-side prep of one core's input map. x_shard [T, C] fp32.
    `shared` caches the (identical) weight arrays across cores."""
    T, C, D = cfg.T, cfg.C, cfg.D
    if not shared:
        shared["w_qkT"] = np.ascontiguousarray(w_qkv[:2 * C].T).astype(NPBF16)
        shared["w_vT"] = np.ascontiguousarray(
            w_qkv[2 * C:3 * C].T).astype(NPBF16)
        shared["w_oT"] = np.ascontiguousarray(w_o.T).astype(NPBF16)
        shared["w_upT"] = np.ascontiguousarray(w_up.T).astype(NPBF16)
        shared["w_downT"] = np.ascontiguousarray(w_down.T).astype(NPBF16)
        shared["nw1"] = attn_norm_w.reshape(1, C).astype(NPBF16)
        shared["nw2"] = ffn_norm_w.reshape(1, C).astype(NPBF16)
        k_idx = np.arange(128)
        shared["trimask"] = (
            k_idx[:, None] <= k_idx[None, :]).astype(NPBF16)
        psw = np.zeros((128, 128), dtype=NPBF16)
        psw[k_idx ^ 32, k_idx] = 1.0  # lhsT[j, p] = 1 iff j == p ^ 32
        shared["pswap"] = psw
    inv = (1.0 / ROPE_THETA ** (np.arange(0, D, 2) / D)).astype(np.float64)
    pos = np.arange(pos0, pos0 + T, dtype=np.float64)
    fr = np.outer(pos, inv)                       # [T, D/2]
    emb = np.concatenate([fr, fr], axis=-1)       # [T, D]
    cosT = np.cos(emb).T.astype(np.float32)       # [D, T]
    sinT = np.sin(emb).T.astype(np.float32)
    nsinT = sinT.copy()
    nsinT[:D // 2] *= -1.0
    reps = 128 // D
    nsin2 = np.tile(nsinT, (reps, 1))
    perm = np.arange(128) ^ 32
    s2 = nsin2[perm]          # s2[p] = nsin2[p ^ 32]
    xt_host = np.ascontiguousarray(x_shard.T)
    return {
        "xT": xt_host.astype(np.float32),
        "xbT": xt_host.astype(NPBF16),
        "cosT2": np.tile(cosT, (reps, 1)).astype(NPBF16),
        "nsinT2": s2.astype(NPBF16),
        **shared,
    }


def kernel(x, attn_norm_w, ffn_norm_w, w_qkv, w_o, w_up, w_down,
           _trace=False, _tmpdir=None):
    x = np.asarray(x, dtype=np.float32)
    attn_norm_w = np.asarray(attn_norm_w, dtype=np.float32)
    ffn_norm_w = np.asarray(ffn_norm_w, dtype=np.float32)
    w_qkv = np.asarray(w_qkv, dtype=np.float32)
    w_o = np.asarray(w_o, dtype=np.float32)
    w_up = np.asarray(w_up, dtype=np.float32)
    w_down = np.asarray(w_down, dtype=np.float32)

    B, S, C = x.shape
    cfg = Cfg(T=512, C=C, H=C // 64, D=64, I=2 * C)
    n_blocks = S // cfg.T
    assert B * n_blocks == 8

    nc = build_program(cfg)

    shared = {}
    in_maps = []
    for core in range(8):
        b, blk = divmod(core, n_blocks)
        sl = slice(blk * cfg.T, (blk + 1) * cfg.T)
        in_maps.append(make_core_inputs(
            cfg, x[b, sl], w_qkv, w_o, w_up, w_down,
            attn_norm_w, ffn_norm_w, pos0=blk * cfg.T, shared=shared,
        ))

    res = run_bass_kernel_spmd(
        nc, in_maps, core_ids=list(range(8)),
        trace=_trace, tmpdir=_tmpdir,
    )

    out = np.empty((B, S, C), dtype=np.float32)
    for core in range(8):
        b, blk = divmod(core, n_blocks)
        sl = slice(blk * cfg.T, (blk + 1) * cfg.T)
        out[b, sl] = res.results[core]["outT"].T
    kernel.last_result = res
    return out
